# revision 1
# baseline (speedup 1.0000x reference)
"""Trainium2 Bass kernel: CNN-feature SoftDTW few-shot classifier.

Computes, for Q=100 query sequences and S=25 support sequences (T=128 steps,
D=2048 features): pairwise squared-euclidean cost matrices, soft-DTW alignment
cost per (query, support) pair, then per-class mean distances -> logits.

Key numerical fact: with gamma=0.1 and cost magnitudes ~4096, the reference's
fp32 softmin is bitwise the hard min (exp((m-x)/gamma) underflows for every
non-minimal branch), so the DP is computed with min/add only. Each DP row is
one `tensor_tensor_scan(op0=min, op1=add)` instruction.

Sharding: data-parallel over queries, 13 per core (Q padded 100->104),
supports replicated. Per core:
  - PE: xy = (-2X)@Y^T in bf16 (16 K-tiles) + fp32r rank-2 update adding
    x2[i] + y2[s,j] exactly -> full cost matrix D in PSUM (fp32).
  - ACT: evacuate PSUM -> SBUF; DMA D to DRAM scratch per query.
  - DMA gather: re-layout D from [i, (s,j)] to [(q,s)-partition, i-window, j].
  - DVE: hard-DTW rows: min(up,diag) + scan, 128 rows x 3 pair-streams.
Host: bf16 packing/transposes, x2/y2 sums, final class-mean logits.
"""

import sys

for _p in ("/opt/trn_rl_repo",):
    if _p not in sys.path:
        sys.path.insert(0, _p)

import numpy as np
import ml_dtypes

# Problem shape (hardcoded: harness runs kernel.py standalone)
Q, S, T, DD = 100, 25, 128, 2048
NCORES = 8
QC = 13                 # queries per core; Q padded to 104
QPAD = QC * NCORES
NK = DD // 128          # 16 bf16 contraction tiles
SJ = S * T              # 3200 = flattened (support, j)
B = QC * S              # 325 pairs per core
# DP pair-tile streams aligned to query boundaries (offset, count):
# a stream's first row can run as soon as its LAST query's cost matrix is
# in DRAM, so query-aligned splits start streams as early as possible.
PT = [(0, 125), (125, 100), (225, 100)]   # q0-4 | q5-8 | q9-12
# 5/4/4 split: stream deps land at ~157/248/338us (query cadence ~23us), so
# the DVE picks up each stream right as its last query's D lands — the final
# stream starts at its data dependency instead of queueing behind stream 1.
W = 16                  # DP row-window per gather DMA
CH = 512                # matmul moving-chunk / PSUM bank width
# all chunks >=256 so the fp32r rank-2 matmul stays at 1 cycle/row
_CW = [512, 512, 512, 512, 512, 384, 256]
CHUNKS = [(sum(_CW[:i]), w) for i, w in enumerate(_CW)]
assert sum(_CW) == SJ
BIG = 1e10

_built = None          # cached compiled Bass program
_last_result = None    # last BassKernelResults (exec_time_ns when traced)
_predicted_ns = None   # Tile cost-model makespan of the per-core program


def _build():
    import concourse.bacc as bacc
    import concourse.mybir as mybir
    import concourse.tile as tile

    f32 = mybir.dt.float32
    f32r = mybir.dt.float32r
    bf16 = mybir.dt.bfloat16
    MIN = mybir.AluOpType.min
    ADD = mybir.AluOpType.add

    global _predicted_ns
    nc = bacc.Bacc("TRN2", debug=False)

    xt_d = nc.dram_tensor("xt", [QC, 128, NK * T], bf16, kind="ExternalInput")
    yt_d = nc.dram_tensor("yt", [128, NK * SJ], bf16, kind="ExternalInput")
    augl_d = nc.dram_tensor("augl", [QC, 2, T], f32r, kind="ExternalInput")
    augr_d = nc.dram_tensor("augr", [2, SJ], f32r, kind="ExternalInput")
    out_d = nc.dram_tensor("out_cd", [QC, S], f32, kind="ExternalOutput")
    # cost matrices staged pair-major: [q, s, i, j] -> window reads are
    # single 3-dim APs with 8KB-contiguous runs per pair
    dsc = nc.dram_tensor("dsc", [QC, S, T, T], f32)
    dsc_p = dsc[:].rearrange("q s i j -> (q s) i j")

    with tile.TileContext(nc) as tc:
        with (
            tc.tile_pool(name="const", bufs=1) as constp,
            tc.tile_pool(name="xq", bufs=2) as xqp,
            tc.tile_pool(name="augq", bufs=2) as augqp,
            tc.tile_pool(name="psum", bufs=8, space="PSUM") as psump,
            tc.tile_pool(name="dq", bufs=1) as dqp,
            tc.tile_pool(name="ga", bufs=2) as gap,      # pair-tiles 0 and 2
            tc.tile_pool(name="gb", bufs=2) as gbp,      # pair-tile 1
            tc.tile_pool(name="muda", bufs=2) as mudap,
            tc.tile_pool(name="mudb", bufs=2) as mudbp,
            tc.tile_pool(name="dp", bufs=1) as dpp,
        ):
            # q0's operands first on the ACT queue (ahead of the yt halves).
            xt0_sb = xqp.tile([128, NK * T], bf16, tag="xt")
            nc.scalar.dma_start(xt0_sb[:], xt_d[0])
            augl0_sb = augqp.tile([2, T], f32r, tag="augl")
            nc.scalar.dma_start(augl0_sb[:], augl_d[0])

            # Resident Y^T (bf16), per K-tile so q0 starts after ~800KB, and
            # alternated across the two physical HWDGE rings (SP + ACT FIFOs)
            # so the 13MB load streams at double the single-FIFO rate.
            yt_sb = constp.tile([128, NK * SJ], bf16)
            for k in range(NK):
                qeng = nc.sync if k % 2 == 0 else nc.scalar
                qeng.dma_start(yt_sb[:, k * SJ:(k + 1) * SJ],
                               yt_d[:, k * SJ:(k + 1) * SJ])
            augr_sb = constp.tile([2, SJ], f32r)
            nc.sync.dma_start(augr_sb[:], augr_d[:])

            # ---- Stage A: cost matrices, one query at a time ----
            for q in range(QC):
                if q == 0:
                    xt_sb, augl_sb = xt0_sb, augl0_sb
                else:
                    # scalar (ACT) HWDGE queue: out of the SP FIFO.
                    xt_sb = xqp.tile([128, NK * T], bf16, tag="xt")
                    nc.scalar.dma_start(xt_sb[:], xt_d[q])
                    augl_sb = augqp.tile([2, T], f32r, tag="augl")
                    nc.scalar.dma_start(augl_sb[:], augl_d[q])

                dq_sb = dqp.tile([128, SJ], f32, tag="dq")
                if q == 0:
                    # k-OUTER while the 16 yt K-tile loads stream in: every
                    # arriving K-tile feeds all 7 chunks (7 PSUM banks live),
                    # so q0's matrix completes with the prologue instead of
                    # 22us after it. Per-cell accumulation order is unchanged.
                    pss = []
                    for _ci in range(len(CHUNKS)):
                        ps_q0 = psump.tile([128, CH], f32, tag="ps")
                        pss.append(ps_q0)
                    for k in range(NK):
                        for ci, (c0, cw) in enumerate(CHUNKS):
                            nc.tensor.matmul(
                                pss[ci][:, :cw],
                                xt_sb[:, k * T:(k + 1) * T],
                                yt_sb[:, k * SJ + c0: k * SJ + c0 + cw],
                                start=(k == 0),
                                stop=False,
                            )
                    for ci, (c0, cw) in enumerate(CHUNKS):
                        nc.tensor.matmul(
                            pss[ci][:, :cw],
                            augl_sb[:, :],
                            augr_sb[:, c0:c0 + cw],
                            start=False,
                            stop=True,
                        )
                        nc.scalar.copy(dq_sb[:, c0:c0 + cw], pss[ci][:, :cw])
                        nc.sync.dma_start(
                            dsc[q, c0 // T:(c0 + cw) // T]
                            .rearrange("s i j -> i s j"),
                            dq_sb[:, c0:c0 + cw]
                            .rearrange("i (s j) -> i s j", j=T))
                else:
                    for c0, cw in CHUNKS:
                        ps = psump.tile([128, CH], f32, tag="ps")
                        for k in range(NK):
                            nc.tensor.matmul(
                                ps[:, :cw],
                                xt_sb[:, k * T:(k + 1) * T],
                                yt_sb[:, k * SJ + c0: k * SJ + c0 + cw],
                                start=(k == 0),
                                stop=False,
                            )
                        # rank-2 fp32 update: + ones*y2[s,j] + x2[i]*ones
                        nc.tensor.matmul(
                            ps[:, :cw],
                            augl_sb[:, :],
                            augr_sb[:, c0:c0 + cw],
                            start=False,
                            stop=True,
                        )
                        nc.scalar.copy(dq_sb[:, c0:c0 + cw], ps[:, :cw])
                        # per-chunk dsc write (chunk widths are whole
                        # s-blocks): the last piece lands ~3.5us after the
                        # last evac instead of a 5us whole-query DMA.
                        nc.sync.dma_start(
                            dsc[q, c0 // T:(c0 + cw) // T]
                            .rearrange("s i j -> i s j"),
                            dq_sb[:, c0:c0 + cw]
                            .rearrange("i (s j) -> i s j", j=T))

            # ---- Stage B: hard-DTW wavefront, 3 batched pair-tiles ----
            out_flat = out_d[:].rearrange("q s -> (q s)")
            for pt, (p0, np_) in enumerate(PT):
                eng = nc.vector  # Pool lacks 2-input TensorTensor on TRN2
                gpool = gbp if pt == 1 else gap
                mudp = mudbp if pt == 1 else mudap
                qa, qb = p0 // S, (p0 + np_ - 1) // S  # query range (aligned)

                r_a = dpp.tile([128, T + 4], f32, tag=f"ra{pt}")
                r_b = dpp.tile([128, T + 4], f32, tag=f"rb{pt}")
                # row 0: [0, BIG, BIG, ...]; r_b border col = BIG.
                # memsets on Pool: keeps them off the DVE critical chain.
                nc.gpsimd.memset(r_a[:np_, 1:T + 1], BIG)
                nc.gpsimd.memset(r_a[:np_, 0:1], 0.0)
                nc.gpsimd.memset(r_b[:np_, 0:1], BIG)

                g_tiles = {}
                for i in range(T):
                    if i % W == 0:
                        g_t = gpool.tile([128, W * T], f32, tag=f"g{pt % 2}")
                        g_tiles[i // W] = g_t
                        # One DMA per window (full SDMA-engine spread).
                        # Pool/SWDGE: idle sequencer, not paced by ACT/SP.
                        # Window 0 of the last stream splits off the final
                        # query so the earlier queries prefetch while q12's
                        # matrix is still being written.
                        if pt == len(PT) - 1 and i == 0:
                            cut = np_ - S
                            nc.gpsimd.dma_start(
                                g_t[:cut, :].rearrange("p (w j) -> p w j", j=T),
                                dsc_p[p0:p0 + cut, i:i + W, :],
                            )
                            nc.gpsimd.dma_start(
                                g_t[cut:np_, :].rearrange(
                                    "p (w j) -> p w j", j=T),
                                dsc_p[p0 + cut:p0 + np_, i:i + W, :],
                            )
                        else:
                            nc.gpsimd.dma_start(
                                g_t[:np_, :].rearrange("p (w j) -> p w j", j=T),
                                dsc_p[p0:p0 + np_, i:i + W, :],
                            )
                    g_t = g_tiles[i // W]
                    prev, cur = (r_a, r_b) if i % 2 == 0 else (r_b, r_a)
                    mud = mudp.tile([128, T], f32, tag=f"m{pt % 2}")
                    eng.tensor_tensor(
                        mud[:np_, :], prev[:np_, 1:T + 1], prev[:np_, 0:T], MIN)
                    eng.tensor_tensor_scan(
                        cur[:np_, 1:T + 1], mud[:np_, :],
                        g_t[:np_, (i % W) * T:(i % W + 1) * T],
                        BIG, MIN, ADD)
                    if i == 0:
                        # row-0 buffer becomes an interior row: border 0 -> BIG
                        eng.memset(prev[:np_, 0:1], BIG)

                final = r_b if T % 2 == 1 else r_a  # T=128 even -> last cur=r_a
                nc.sync.dma_start(out_flat[p0:p0 + np_], final[:np_, T:T + 1])

    ents = getattr(tc, "_perfetto_entries", None)
    if ents:
        _predicted_ns = int(max(e[2] for e in ents))
    nc.compile()
    return nc


def _pack_inputs(X, Yf):
    """Host-side packing into the exact SBUF layouts the kernel DMAs 1:1."""
    bf = ml_dtypes.bfloat16
    # xt[c]: [QC, 128(dk), NK*T] = bf16(-2*X)^T, K-tile-major free dim
    Xp = np.zeros((QPAD, T, DD), np.float32)
    Xp[:Q] = X
    xtq = np.ascontiguousarray(
        (-2.0 * Xp).astype(bf).transpose(0, 2, 1)        # [QPAD, DD, T]
        .reshape(QPAD, NK, 128, T).transpose(0, 2, 1, 3)  # [QPAD, 128, NK, T]
        .reshape(QPAD, 128, NK * T))
    # yt: [128(dk), NK*SJ] = bf16(Y)^T
    yt = np.ascontiguousarray(
        Yf.astype(bf).transpose(2, 0, 1)                 # [DD, S, T]
        .reshape(NK, 128, SJ).transpose(1, 0, 2)         # [128, NK, SJ]
        .reshape(128, NK * SJ))
    # exact fp32 norms
    x2 = np.einsum("qtd,qtd->qt", Xp, Xp, dtype=np.float32)  # [QPAD, T]
    y2 = np.einsum("std,std->st", Yf, Yf, dtype=np.float32)  # [S, T]
    augl = np.zeros((QPAD, 2, T), np.float32)
    augl[:, 0, :] = 1.0
    augl[:, 1, :] = x2
    augr = np.zeros((2, SJ), np.float32)
    augr[0] = y2.reshape(SJ)
    augr[1] = 1.0
    return xtq, yt, augl, augr


def kernel(support_features, support_labels, target_features, n_classes):
    global _built
    from concourse.bass_utils import run_bass_kernel_spmd

    X = np.asarray(target_features, dtype=np.float32)
    Yf = np.asarray(support_features, dtype=np.float32)
    labels = np.asarray(support_labels)
    ncls = int(np.asarray(n_classes))
    assert X.shape == (Q, T, DD) and Yf.shape == (S, T, DD), (
        f"kernel compiled for fixed shapes; got {X.shape}, {Yf.shape}")

    xtq, yt, augl, augr = _pack_inputs(X, Yf)

    if _built is None:
        _built = _build()
    nc = _built

    in_maps = [
        {
            "xt": np.ascontiguousarray(xtq[c * QC:(c + 1) * QC]),
            "yt": yt,
            "augl": np.ascontiguousarray(augl[c * QC:(c + 1) * QC]),
            "augr": augr,
        }
        for c in range(NCORES)
    ]
    res = run_bass_kernel_spmd(nc, in_maps, list(range(NCORES)))
    global _last_result
    _last_result = res
    cum = np.concatenate([res.results[c]["out_cd"] for c in range(NCORES)])[:Q]

    onehot = (labels[:, None] == np.arange(ncls)[None, :]).astype(np.float32)
    counts = np.maximum(onehot.sum(axis=0), 1.0).astype(np.float32)
    logits = -(cum.astype(np.float32) @ onehot) / counts
    return logits.astype(np.float32)



# revision 2
# speedup vs baseline: 1.7664x; 1.7664x over previous
"""Trainium2 Bass kernel: CNN-feature SoftDTW few-shot classifier (v2).

Computes, for Q=100 query sequences and S=25 support sequences (T=128 steps,
D=2048 features): pairwise squared-euclidean cost matrices, soft-DTW alignment
cost per (query, support) pair, then per-class mean distances -> logits.

Key numerical fact: with gamma=0.1 and cost magnitudes ~4096, the reference's
fp32 softmin is bitwise the hard min (exp((m-x)/gamma) underflows for every
non-minimal branch), so the DP is computed with min/add only. Each DP row is
one `tensor_tensor_scan(op0=min, op1=add)` instruction.

v2 changes vs the 385us baseline:
  - fp8e4m3 DoubleRow matmul (0.5 cy/row, K=256/instr) for xy: PE work for
    the 2048-deep contraction drops 4x vs bf16. Quantization error on the
    DTW path sum is ~1e-4 of the output scale (threshold 2e-2).
  - k-outer matmul order for EVERY query (7 PSUM banks live) so each fp8
    weight tile is loaded once per 3200-column sweep.
  - cost matrices staged in DRAM as bf16 (+-8 absolute per ~4096-cell):
    halves the dsc round-trip DMA. DVE scan reads bf16 data1 directly.
  - DP pair streams split 3/5/5 queries so the wavefront starts after only
    3 query matrices instead of 5.

Sharding: data-parallel over queries, 13 per core (Q padded 100->104),
supports replicated. Host: fp8 packing/transposes, x2/y2 sums, final
class-mean logits.
"""

import sys

for _p in ("/opt/trn_rl_repo",):
    if _p not in sys.path:
        sys.path.insert(0, _p)

import numpy as np
import ml_dtypes

# Problem shape (hardcoded: harness runs kernel.py standalone)
Q, S, T, DD = 100, 25, 128, 2048
NCORES = 8
QC = 13                 # queries per core; Q padded to 104
QPAD = QC * NCORES
NKK = DD // 256         # 8 fp8 DoubleRow contraction tiles (256 deep each)
SJ = S * T              # 3200 = flattened (support, j)
B = QC * S              # 325 pairs per core
# DP pair-tile streams aligned to query boundaries (offset, count):
# a stream's first row can run as soon as its LAST query's cost matrix is
# in DRAM. 3/5/5 queries: the DVE wavefront starts at q2 instead of q4.
PT = [(0, 75), (75, 125), (200, 125)]   # q0-2 | q3-7 | q8-12
W = 16                  # DP row-window per gather DMA
CH = 512                # matmul moving-chunk / PSUM bank width
# all chunks >=256 so the fp32r rank-2 matmul stays at 1 cycle/row
_CW = [512, 512, 512, 512, 512, 384, 256]
CHUNKS = [(sum(_CW[:i]), w) for i, w in enumerate(_CW)]
assert sum(_CW) == SJ
BIG = 1e10

_built = None          # cached compiled Bass program
_last_result = None    # last BassKernelResults (exec_time_ns when traced)
_predicted_ns = None   # Tile cost-model makespan of the per-core program


def _build():
    import concourse.bacc as bacc
    import concourse.mybir as mybir
    import concourse.tile as tile

    f32 = mybir.dt.float32
    f32r = mybir.dt.float32r
    bf16 = mybir.dt.bfloat16
    fp8 = mybir.dt.float8e4
    MIN = mybir.AluOpType.min
    ADD = mybir.AluOpType.add
    DR = mybir.MatmulPerfMode.DoubleRow

    global _predicted_ns
    nc = bacc.Bacc("TRN2", debug=False)

    xt_d = nc.dram_tensor("xt", [QC, 128, NKK, 2, T], fp8, kind="ExternalInput")
    yt_d = nc.dram_tensor("yt", [128, NKK, 2, SJ], fp8, kind="ExternalInput")
    augl_d = nc.dram_tensor("augl", [QC, 2, T], f32r, kind="ExternalInput")
    augr_d = nc.dram_tensor("augr", [2, SJ], f32r, kind="ExternalInput")
    out_d = nc.dram_tensor("out_cd", [QC, S], f32, kind="ExternalOutput")
    # cost matrices staged pair-major: [q, s, i, j] -> window reads are
    # single 3-dim APs with 4KB-contiguous runs per pair
    dsc = nc.dram_tensor("dsc", [QC, S, T, T], bf16)
    dsc_p = dsc[:].rearrange("q s i j -> (q s) i j")

    with tile.TileContext(nc) as tc:
        with (
            tc.tile_pool(name="const", bufs=1) as constp,
            tc.tile_pool(name="xq", bufs=2) as xqp,
            tc.tile_pool(name="augq", bufs=2) as augqp,
            tc.tile_pool(name="psum", bufs=8, space="PSUM") as psump,
            tc.tile_pool(name="dq", bufs=2) as dqp,
            tc.tile_pool(name="ga", bufs=2) as gap,      # pair-tiles 0 and 2
            tc.tile_pool(name="gb", bufs=2) as gbp,      # pair-tile 1
            tc.tile_pool(name="muda", bufs=2) as mudap,
            tc.tile_pool(name="mudb", bufs=2) as mudbp,
            tc.tile_pool(name="dp", bufs=1) as dpp,
        ):
            # q0's operands first on the ACT queue (ahead of the yt pieces).
            xt0_sb = xqp.tile([128, NKK, 2, T], fp8, tag="xt")
            nc.scalar.dma_start(xt0_sb[:], xt_d[0])
            augl0_sb = augqp.tile([2, T], f32r, tag="augl")
            nc.scalar.dma_start(augl0_sb[:], augl_d[0])

            # Resident Y^T (fp8), per K-tile so q0 starts after ~800KB, and
            # alternated across the two physical HWDGE rings (SP + ACT FIFOs).
            yt_sb = constp.tile([128, NKK, 2, SJ], fp8)
            for k in range(NKK):
                qeng = nc.sync if k % 2 == 0 else nc.scalar
                qeng.dma_start(yt_sb[:, k, :, :], yt_d[:, k, :, :])
            augr_sb = constp.tile([2, SJ], f32r)
            nc.sync.dma_start(augr_sb[:], augr_d[:])

            # ---- Stage A: cost matrices, one query at a time (k-outer) ----
            xt_tiles = {0: xt0_sb}
            augl_tiles = {0: augl0_sb}
            for q in range(QC):
                xt_sb, augl_sb = xt_tiles.pop(q), augl_tiles.pop(q)
                dq_sb = dqp.tile([128, SJ], bf16, tag="dq")
                # k-OUTER: each fp8 weight tile streams all 7 chunks (7 PSUM
                # banks live), so weight loads amortize 7x; for q0 every
                # arriving yt K-tile immediately feeds all chunks, so q0's
                # matrix completes with the prologue.
                pss = []
                for _ci in range(len(CHUNKS)):
                    ps_t = psump.tile([128, CH], f32, tag="ps")
                    pss.append(ps_t)
                for kk in range(NKK):
                    for ci, (c0, cw) in enumerate(CHUNKS):
                        nc.tensor.matmul(
                            pss[ci][:, :cw],
                            xt_sb[:, kk, :, :],
                            yt_sb[:, kk, :, c0:c0 + cw],
                            start=(kk == 0),
                            stop=False,
                            perf_mode=DR,
                        )
                for ci, (c0, cw) in enumerate(CHUNKS):
                    # rank-2 fp32 update: + ones*y2[s,j] + x2[i]*ones
                    nc.tensor.matmul(
                        pss[ci][:, :cw],
                        augl_sb[:, :],
                        augr_sb[:, c0:c0 + cw],
                        start=False,
                        stop=True,
                    )
                    nc.scalar.copy(dq_sb[:, c0:c0 + cw], pss[ci][:, :cw])
                    if ci == 0 and q + 1 < QC:
                        # next query's operands: issued here so the ACT-ring
                        # DMA lands well before q+1's first matmul needs it
                        # (instead of queueing behind all 7 evacs).
                        xt_sb_n = xqp.tile([128, NKK, 2, T], fp8, tag="xt")
                        nc.scalar.dma_start(xt_sb_n[:], xt_d[q + 1])
                        augl_sb_n = augqp.tile([2, T], f32r, tag="augl")
                        nc.scalar.dma_start(augl_sb_n[:], augl_d[q + 1])
                        xt_tiles[q + 1] = xt_sb_n
                        augl_tiles[q + 1] = augl_sb_n
                    # per-chunk dsc write (chunk widths are whole s-blocks)
                    nc.sync.dma_start(
                        dsc[q, c0 // T:(c0 + cw) // T]
                        .rearrange("s i j -> i s j"),
                        dq_sb[:, c0:c0 + cw]
                        .rearrange("i (s j) -> i s j", j=T))

            # ---- Stage B: hard-DTW wavefront, 3 batched pair-tiles ----
            out_flat = out_d[:].rearrange("q s -> (q s)")
            for pt, (p0, np_) in enumerate(PT):
                eng = nc.vector  # Pool lacks 2-input TensorTensor on TRN2
                gpool = gbp if pt == 1 else gap
                mudp = mudbp if pt == 1 else mudap

                r_a = dpp.tile([128, T + 4], f32, tag=f"ra{pt}")
                r_b = dpp.tile([128, T + 4], f32, tag=f"rb{pt}")
                # row 0: [0, BIG, BIG, ...]; r_b border col = BIG.
                # memsets on Pool: keeps them off the DVE critical chain.
                nc.gpsimd.memset(r_a[:np_, 1:T + 1], BIG)
                nc.gpsimd.memset(r_a[:np_, 0:1], 0.0)
                nc.gpsimd.memset(r_b[:np_, 0:1], BIG)

                g_tiles = {}
                for i in range(T):
                    if i % W == 0:
                        g_t = gpool.tile([128, W * T], bf16, tag=f"g{pt % 2}")
                        g_tiles[i // W] = g_t
                        # One DMA per window (full SDMA-engine spread).
                        # Pool/SWDGE: idle sequencer, not paced by ACT/SP.
                        # Window 0 splits off the final query so the earlier
                        # queries prefetch while the last matrix is still
                        # being written.
                        if pt >= 1 and i == 0:
                            cut = np_ - S
                            nc.gpsimd.dma_start(
                                g_t[:cut, :].rearrange("p (w j) -> p w j", j=T),
                                dsc_p[p0:p0 + cut, i:i + W, :],
                            )
                            nc.gpsimd.dma_start(
                                g_t[cut:np_, :].rearrange(
                                    "p (w j) -> p w j", j=T),
                                dsc_p[p0 + cut:p0 + np_, i:i + W, :],
                            )
                        else:
                            nc.gpsimd.dma_start(
                                g_t[:np_, :].rearrange("p (w j) -> p w j", j=T),
                                dsc_p[p0:p0 + np_, i:i + W, :],
                            )
                    g_t = g_tiles[i // W]
                    prev, cur = (r_a, r_b) if i % 2 == 0 else (r_b, r_a)
                    mud = mudp.tile([128, T], f32, tag=f"m{pt % 2}")
                    eng.tensor_tensor(
                        mud[:np_, :], prev[:np_, 1:T + 1], prev[:np_, 0:T], MIN)
                    eng.tensor_tensor_scan(
                        cur[:np_, 1:T + 1], mud[:np_, :],
                        g_t[:np_, (i % W) * T:(i % W + 1) * T],
                        BIG, MIN, ADD)
                    if i == 0:
                        # row-0 buffer becomes an interior row: border 0 -> BIG
                        eng.memset(prev[:np_, 0:1], BIG)

                final = r_b if T % 2 == 1 else r_a  # T=128 even -> last cur=r_a
                nc.sync.dma_start(out_flat[p0:p0 + np_], final[:np_, T:T + 1])

    ents = getattr(tc, "_perfetto_entries", None)
    if ents:
        _predicted_ns = int(max(e[2] for e in ents))
    nc.compile()
    return nc


def _pack_inputs(X, Yf):
    """Host-side packing into the exact SBUF layouts the kernel DMAs 1:1."""
    f8 = ml_dtypes.float8_e4m3
    # xt: [QPAD, 128(dk), NKK, 2, T] = fp8(-2*X)^T, DoubleRow slab layout
    # contraction index d = kk*256 + slab*128 + dk
    Xp = np.zeros((QPAD, T, DD), np.float32)
    Xp[:Q] = X
    xtq = np.ascontiguousarray(
        (-2.0 * Xp).astype(f8).transpose(0, 2, 1)        # [QPAD, DD, T]
        .reshape(QPAD, NKK, 2, 128, T).transpose(0, 3, 1, 2, 4))
    # yt: [128(dk), NKK, 2, SJ] = fp8(Y)^T
    yt = np.ascontiguousarray(
        Yf.astype(f8).transpose(2, 0, 1)                 # [DD, S, T]
        .reshape(NKK, 2, 128, SJ).transpose(2, 0, 1, 3))
    # exact fp32 norms
    x2 = np.einsum("qtd,qtd->qt", Xp, Xp, dtype=np.float32)  # [QPAD, T]
    y2 = np.einsum("std,std->st", Yf, Yf, dtype=np.float32)  # [S, T]
    augl = np.zeros((QPAD, 2, T), np.float32)
    augl[:, 0, :] = 1.0
    augl[:, 1, :] = x2
    augr = np.zeros((2, SJ), np.float32)
    augr[0] = y2.reshape(SJ)
    augr[1] = 1.0
    return xtq, yt, augl, augr


def kernel(support_features, support_labels, target_features, n_classes):
    global _built
    from concourse.bass_utils import run_bass_kernel_spmd

    X = np.asarray(target_features, dtype=np.float32)
    Yf = np.asarray(support_features, dtype=np.float32)
    labels = np.asarray(support_labels)
    ncls = int(np.asarray(n_classes))
    assert X.shape == (Q, T, DD) and Yf.shape == (S, T, DD), (
        f"kernel compiled for fixed shapes; got {X.shape}, {Yf.shape}")

    xtq, yt, augl, augr = _pack_inputs(X, Yf)

    if _built is None:
        _built = _build()
    nc = _built

    in_maps = [
        {
            "xt": np.ascontiguousarray(xtq[c * QC:(c + 1) * QC]),
            "yt": yt,
            "augl": np.ascontiguousarray(augl[c * QC:(c + 1) * QC]),
            "augr": augr,
        }
        for c in range(NCORES)
    ]
    res = run_bass_kernel_spmd(nc, in_maps, list(range(NCORES)))
    global _last_result
    _last_result = res
    cum = np.concatenate([res.results[c]["out_cd"] for c in range(NCORES)])[:Q]

    onehot = (labels[:, None] == np.arange(ncls)[None, :]).astype(np.float32)
    counts = np.maximum(onehot.sum(axis=0), 1.0).astype(np.float32)
    logits = -(cum.astype(np.float32) @ onehot) / counts
    return logits.astype(np.float32)


# revision 3
# speedup vs baseline: 1.8616x; 1.0539x over previous
"""Trainium2 Bass kernel: CNN-feature SoftDTW few-shot classifier (v2).

Computes, for Q=100 query sequences and S=25 support sequences (T=128 steps,
D=2048 features): pairwise squared-euclidean cost matrices, soft-DTW alignment
cost per (query, support) pair, then per-class mean distances -> logits.

Key numerical fact: with gamma=0.1 and cost magnitudes ~4096, the reference's
fp32 softmin is bitwise the hard min (exp((m-x)/gamma) underflows for every
non-minimal branch), so the DP is computed with min/add only. Each DP row is
one `tensor_tensor_scan(op0=min, op1=add)` instruction.

v3 changes vs the 385us baseline:
  - fp8e4m3 DoubleRow matmul (0.5 cy/row, K=256/instr) for xy: PE work for
    the 2048-deep contraction drops 4x vs bf16. Quantization error on the
    DTW path sum is ~1e-4 of the output scale (threshold 2e-2).
  - k-outer matmul order for EVERY query (7 PSUM banks live) so each fp8
    weight tile is loaded once per 3200-column sweep.
  - cost matrices staged in DRAM as bf16 (+-8 absolute per ~4096-cell):
    halves the dsc round-trip DMA. DVE scan reads bf16 data1 directly.
  - DP pair streams split 3/5/5 queries so the wavefront starts after only
    3 query matrices instead of 5.

Sharding: data-parallel over queries, 13 per core (Q padded 100->104),
supports replicated. Host: fp8 packing/transposes, x2/y2 sums, final
class-mean logits.
"""

import sys

for _p in ("/opt/trn_rl_repo",):
    if _p not in sys.path:
        sys.path.insert(0, _p)

import numpy as np
import ml_dtypes

# Problem shape (hardcoded: harness runs kernel.py standalone)
Q, S, T, DD = 100, 25, 128, 2048
NCORES = 8
QC = 13                 # queries per core; Q padded to 104
QPAD = QC * NCORES
NKK = DD // 256         # 8 fp8 DoubleRow contraction tiles (256 deep each)
SJ = S * T              # 3200 = flattened (support, j)
B = QC * S              # 325 pairs per core
# DP pair-tile streams aligned to query boundaries (offset, count):
# a stream's first row can run as soon as its LAST query's cost matrix is
# in DRAM. 3/5/5 queries: the DVE wavefront starts at q2 instead of q4.
PT = [(0, 75), (75, 125), (200, 125)]   # q0-2 | q3-7 | q8-12
# stage-A processing order of the 13 query slots. Streams gate on their
# LAST slot: s1 on slot 2, s2 on slot 7, s3 on slot 12.
QORDER = list(range(13))
# DP row-windows per gather DMA: small first window so each stream's
# wavefront starts ~2us earlier (less gather gen+transfer to wait on).
# Stream 2 (middle) uses smaller windows + a single-buffered gather pool:
# its rows are RELEASED gradually, so the greedy scheduler cannot drain
# stream 2 early and leave stream 3 alone (at 66% serial rate) in the tail.
def _wins(first, mid):
    w, i = [(0, first)], first
    while i < T:
        n = min(mid, T - i)
        w.append((i, n))
        i += n
    return w[:len(w)]
WINS_S = {0: _wins(4, 16), 1: _wins(4, 16), 2: _wins(4, 16)}
for _w in WINS_S.values():
    assert sum(n for _, n in _w) == T
CH = 512                # matmul moving-chunk / PSUM bank width
# all chunks >=256 so the fp32r rank-2 matmul stays at 1 cycle/row
_CW = [512, 512, 512, 512, 512, 384, 256]
CHUNKS = [(sum(_CW[:i]), w) for i, w in enumerate(_CW)]
assert sum(_CW) == SJ
BIG = 1e10

_built = None          # cached compiled Bass program
_last_result = None    # last BassKernelResults (exec_time_ns when traced)
_predicted_ns = None   # Tile cost-model makespan of the per-core program


def _build():
    import concourse.bacc as bacc
    import concourse.mybir as mybir
    import concourse.tile as tile

    f32 = mybir.dt.float32
    f32r = mybir.dt.float32r
    bf16 = mybir.dt.bfloat16
    fp8 = mybir.dt.float8e4
    MIN = mybir.AluOpType.min
    ADD = mybir.AluOpType.add
    DR = mybir.MatmulPerfMode.DoubleRow

    global _predicted_ns
    nc = bacc.Bacc("TRN2", debug=False)

    xt_d = nc.dram_tensor("xt", [QC, 128, NKK, 2, T], fp8, kind="ExternalInput")
    yt_d = nc.dram_tensor("yt", [128, NKK, 2, SJ], fp8, kind="ExternalInput")
    # aux DoubleRow rows (K_p=2, 2 slabs = 4 contraction rows):
    #   (k0,s0): 64 * 64            -> +4096 (exact mean of x2+y2)
    #   (k1,s0): rx[i]/8 * 8        -> +(x2[i]-2048) residual
    #   (k0,s1): 8 * ry[sj]/8       -> +(y2[sj]-2048) residual
    #   (k1,s1): 0
    xa_d = nc.dram_tensor("xa", [QC, 2, 2, T], fp8, kind="ExternalInput")
    ya_d = nc.dram_tensor("ya", [2, 2, SJ], fp8, kind="ExternalInput")
    out_d = nc.dram_tensor("out_cd", [QC, S], f32, kind="ExternalOutput")
    # cost matrices staged pair-major: [q, s, i, j] -> window reads are
    # single 3-dim APs with 4KB-contiguous runs per pair
    dsc = nc.dram_tensor("dsc", [QC, S, T, T], bf16)
    dsc_p = dsc[:].rearrange("q s i j -> (q s) i j")

    with tile.TileContext(nc) as tc:
        with (
            tc.tile_pool(name="const", bufs=1) as constp,
            tc.tile_pool(name="xq", bufs=2) as xqp,
            tc.tile_pool(name="augq", bufs=2) as augqp,
            tc.tile_pool(name="psum", bufs=8, space="PSUM") as psump,
            tc.tile_pool(name="dq", bufs=2) as dqp,
            tc.tile_pool(name="ga", bufs=2) as gap,
            tc.tile_pool(name="gb", bufs=1) as gbp,
            tc.tile_pool(name="gc", bufs=2) as gcp,
            tc.tile_pool(name="muda", bufs=2) as mudap,
            tc.tile_pool(name="mudb", bufs=2) as mudbp,
            tc.tile_pool(name="mudc", bufs=2) as mudcp,
            tc.tile_pool(name="dp", bufs=1) as dpp,
        ):
            # q0's operands first on the ACT queue (ahead of the yt pieces).
            xt0_sb = xqp.tile([128, NKK, 2, T], fp8, tag="xt")
            nc.scalar.dma_start(xt0_sb[:], xt_d[QORDER[0]])
            xa0_sb = augqp.tile([2, 2, T], fp8, tag="xa")
            nc.scalar.dma_start(xa0_sb[:], xa_d[QORDER[0]])

            # Resident Y^T (fp8), per K-tile so q0 starts after ~800KB,
            # spread across 3 DMA queues (SP + ACT + Pool/SWDGE) so all of
            # yt lands within ~7us. The Pool sequencer is idle until the
            # wavefront starts, so the SWDGE generation there is free.
            yt_sb = constp.tile([128, NKK, 2, SJ], fp8)
            for k in range(NKK):
                qeng = [nc.sync, nc.scalar, nc.gpsimd][k % 3]
                qeng.dma_start(yt_sb[:, k, :, :], yt_d[:, k, :, :])
            ya_sb = constp.tile([2, 2, SJ], fp8)
            nc.sync.dma_start(ya_sb[:], ya_d[:])

            # ---- Stage A: cost matrices, one query at a time (k-outer) ----
            xt_tiles = {QORDER[0]: xt0_sb}
            xa_tiles = {QORDER[0]: xa0_sb}
            for qi, q in enumerate(QORDER):
                xt_sb, xa_sb = xt_tiles.pop(q), xa_tiles.pop(q)
                dq_sb = dqp.tile([128, SJ], bf16, tag="dq")
                # k-OUTER: each fp8 weight tile streams all 7 chunks (7 PSUM
                # banks live), so weight loads amortize 7x; for q0 every
                # arriving yt K-tile immediately feeds all chunks, so q0's
                # matrix completes with the prologue.
                pss = []
                for _ci in range(len(CHUNKS)):
                    ps_t = psump.tile([128, CH], f32, tag="ps")
                    pss.append(ps_t)
                for kk in range(NKK):
                    for ci, (c0, cw) in enumerate(CHUNKS):
                        nc.tensor.matmul(
                            pss[ci][:, :cw],
                            xt_sb[:, kk, :, :],
                            yt_sb[:, kk, :, c0:c0 + cw],
                            start=(kk == 0),
                            stop=False,
                            perf_mode=DR,
                        )
                for ci, (c0, cw) in enumerate(CHUNKS):
                    # fp8 aux rank-4: + 4096 + rx[i] + ry[s,j]
                    nc.tensor.matmul(
                        pss[ci][:, :cw],
                        xa_sb[:, :, :],
                        ya_sb[:, :, c0:c0 + cw],
                        start=False,
                        stop=True,
                        perf_mode=DR,
                    )
                    nc.scalar.copy(dq_sb[:, c0:c0 + cw], pss[ci][:, :cw])
                    if ci == 0 and qi + 1 < QC:
                        # next query's operands on the SP ring, emitted ahead
                        # of this query's dsc writes: keeps the ACT queue a
                        # pure evac chain (the gather latency of gate queries
                        # depends on it) and still lands with ~5 chunks of
                        # margin before q+1's first matmul.
                        qn = QORDER[qi + 1]
                        xt_sb_n = xqp.tile([128, NKK, 2, T], fp8, tag="xt")
                        nc.sync.dma_start(xt_sb_n[:], xt_d[qn])
                        xa_sb_n = augqp.tile([2, 2, T], fp8, tag="xa")
                        nc.sync.dma_start(xa_sb_n[:], xa_d[qn])
                        xt_tiles[qn] = xt_sb_n
                        xa_tiles[qn] = xa_sb_n
                    # per-chunk dsc write (chunk widths are whole s-blocks)
                    nc.sync.dma_start(
                        dsc[q, c0 // T:(c0 + cw) // T]
                        .rearrange("s i j -> i s j"),
                        dq_sb[:, c0:c0 + cw]
                        .rearrange("i (s j) -> i s j", j=T))

            # ---- Stage B: hard-DTW wavefront, 3 batched pair-tiles ----
            out_flat = out_d[:].rearrange("q s -> (q s)")
            for pt, (p0, np_) in enumerate(PT):
                eng = nc.vector  # Pool lacks 2-input TensorTensor on TRN2
                gpool = [gap, gbp, gcp][pt]
                mudp = [mudap, mudbp, mudcp][pt]

                r_a = dpp.tile([128, T + 4], f32, tag=f"ra{pt}")
                r_b = dpp.tile([128, T + 4], f32, tag=f"rb{pt}")
                # row 0: [0, BIG, BIG, ...]; r_b border col = BIG.
                # memsets on Pool: keeps them off the DVE critical chain.
                nc.gpsimd.memset(r_a[:np_, 1:T + 1], BIG)
                nc.gpsimd.memset(r_a[:np_, 0:1], 0.0)
                nc.gpsimd.memset(r_b[:np_, 0:1], BIG)

                WINS = WINS_S[pt]
                win_of = {}
                for wi, (w0, wn) in enumerate(WINS):
                    for i in range(w0, w0 + wn):
                        win_of[i] = (wi, w0)
                g_tiles = {}
                for i in range(T):
                    wi, w0 = win_of[i]
                    if i == w0:
                        wn = WINS[wi][1]
                        g_t = gpool.tile([128, 16 * T], bf16, tag=f"g{pt}")
                        g_tiles[wi] = g_t
                        # One DMA per window (full SDMA-engine spread).
                        # Pool/SWDGE: idle sequencer, not paced by ACT/SP.
                        # Window 0 splits off the final query so the earlier
                        # queries prefetch while the last matrix is still
                        # being written.
                        if pt >= 1 and wi == 0:
                            cut = np_ - S
                            nc.gpsimd.dma_start(
                                g_t[:cut, :wn * T].rearrange(
                                    "p (w j) -> p w j", j=T),
                                dsc_p[p0:p0 + cut, w0:w0 + wn, :],
                            )
                            nc.gpsimd.dma_start(
                                g_t[cut:np_, :wn * T].rearrange(
                                    "p (w j) -> p w j", j=T),
                                dsc_p[p0 + cut:p0 + np_, w0:w0 + wn, :],
                            )
                        else:
                            nc.gpsimd.dma_start(
                                g_t[:np_, :wn * T].rearrange(
                                    "p (w j) -> p w j", j=T),
                                dsc_p[p0:p0 + np_, w0:w0 + wn, :],
                            )
                    g_t = g_tiles[wi]
                    prev, cur = (r_a, r_b) if i % 2 == 0 else (r_b, r_a)
                    mud = mudp.tile([128, T], f32, tag=f"m{pt}")
                    eng.tensor_tensor(
                        mud[:np_, :], prev[:np_, 1:T + 1], prev[:np_, 0:T], MIN)
                    eng.tensor_tensor_scan(
                        cur[:np_, 1:T + 1], mud[:np_, :],
                        g_t[:np_, (i - w0) * T:(i - w0 + 1) * T],
                        BIG, MIN, ADD)
                    if i == 0:
                        # row-0 buffer becomes an interior row: border 0 -> BIG
                        eng.memset(prev[:np_, 0:1], BIG)

                final = r_b if T % 2 == 1 else r_a  # T=128 even -> last cur=r_a
                nc.sync.dma_start(out_flat[p0:p0 + np_], final[:np_, T:T + 1])

    ents = getattr(tc, "_perfetto_entries", None)
    if ents:
        _predicted_ns = int(max(e[2] for e in ents))
    nc.compile()
    return nc


def _pack_inputs(X, Yf):
    """Host-side packing into the exact SBUF layouts the kernel DMAs 1:1."""
    f8 = ml_dtypes.float8_e4m3
    # xt: [QPAD, 128(dk), NKK, 2, T] = fp8(-2*X)^T, DoubleRow slab layout
    # contraction index d = kk*256 + slab*128 + dk
    Xp = np.zeros((QPAD, T, DD), np.float32)
    Xp[:Q] = X
    xtq = np.ascontiguousarray(
        (-2.0 * Xp).astype(f8).transpose(0, 2, 1)        # [QPAD, DD, T]
        .reshape(QPAD, NKK, 2, 128, T).transpose(0, 3, 1, 2, 4))
    # yt: [128(dk), NKK, 2, SJ] = fp8(Y)^T
    yt = np.ascontiguousarray(
        Yf.astype(f8).transpose(2, 0, 1)                 # [DD, S, T]
        .reshape(NKK, 2, 128, SJ).transpose(2, 0, 1, 3))
    # norms folded into fp8 aux DoubleRow rows:
    # x2+y2 = 4096 (exact, 64*64) + rx + ry residuals (fp8/8, err ~+-8)
    x2 = np.einsum("qtd,qtd->qt", Xp, Xp, dtype=np.float32)  # [QPAD, T]
    y2 = np.einsum("std,std->st", Yf, Yf, dtype=np.float32)  # [S, T]
    xa = np.zeros((QPAD, 2, 2, T), f8)
    xa[:, 0, 0, :] = f8(64.0)
    xa[:, 1, 0, :] = ((x2 - 2048.0) / 8.0).astype(f8)
    xa[:, 0, 1, :] = f8(8.0)
    ya = np.zeros((2, 2, SJ), f8)
    ya[0, 0, :] = f8(64.0)
    ya[1, 0, :] = f8(8.0)
    ya[0, 1, :] = ((y2.reshape(SJ) - 2048.0) / 8.0).astype(f8)
    return xtq, yt, xa, ya


def kernel(support_features, support_labels, target_features, n_classes):
    global _built
    from concourse.bass_utils import run_bass_kernel_spmd

    X = np.asarray(target_features, dtype=np.float32)
    Yf = np.asarray(support_features, dtype=np.float32)
    labels = np.asarray(support_labels)
    ncls = int(np.asarray(n_classes))
    assert X.shape == (Q, T, DD) and Yf.shape == (S, T, DD), (
        f"kernel compiled for fixed shapes; got {X.shape}, {Yf.shape}")

    xtq, yt, xa, ya = _pack_inputs(X, Yf)

    if _built is None:
        _built = _build()
    nc = _built

    in_maps = [
        {
            "xt": np.ascontiguousarray(xtq[c * QC:(c + 1) * QC]),
            "yt": yt,
            "xa": np.ascontiguousarray(xa[c * QC:(c + 1) * QC]),
            "ya": ya,
        }
        for c in range(NCORES)
    ]
    res = run_bass_kernel_spmd(nc, in_maps, list(range(NCORES)))
    global _last_result
    _last_result = res
    cum = np.concatenate([res.results[c]["out_cd"] for c in range(NCORES)])[:Q]

    onehot = (labels[:, None] == np.arange(ncls)[None, :]).astype(np.float32)
    counts = np.maximum(onehot.sum(axis=0), 1.0).astype(np.float32)
    logits = -(cum.astype(np.float32) @ onehot) / counts
    return logits.astype(np.float32)


# revision 4
# speedup vs baseline: 2.0410x; 1.0964x over previous
"""Trainium2 Bass kernel: CNN-feature SoftDTW few-shot classifier (v2).

Computes, for Q=100 query sequences and S=25 support sequences (T=128 steps,
D=2048 features): pairwise squared-euclidean cost matrices, soft-DTW alignment
cost per (query, support) pair, then per-class mean distances -> logits.

Key numerical fact: with gamma=0.1 and cost magnitudes ~4096, the reference's
fp32 softmin is bitwise the hard min (exp((m-x)/gamma) underflows for every
non-minimal branch), so the DP is computed with min/add only. Each DP row is
one `tensor_tensor_scan(op0=min, op1=add)` instruction.

v3 changes vs the 385us baseline:
  - fp8e4m3 DoubleRow matmul (0.5 cy/row, K=256/instr) for xy: PE work for
    the 2048-deep contraction drops 4x vs bf16. Quantization error on the
    DTW path sum is ~1e-4 of the output scale (threshold 2e-2).
  - k-outer matmul order for EVERY query (7 PSUM banks live) so each fp8
    weight tile is loaded once per 3200-column sweep.
  - cost matrices staged in DRAM as bf16 (+-8 absolute per ~4096-cell):
    halves the dsc round-trip DMA. DVE scan reads bf16 data1 directly.
  - DP pair streams split 3/5/5 queries so the wavefront starts after only
    3 query matrices instead of 5.

Sharding: data-parallel over queries, 13 per core (Q padded 100->104),
supports replicated. Host: fp8 packing/transposes, x2/y2 sums, final
class-mean logits.
"""

import sys

for _p in ("/opt/trn_rl_repo",):
    if _p not in sys.path:
        sys.path.insert(0, _p)

import numpy as np
import ml_dtypes

# Problem shape (hardcoded: harness runs kernel.py standalone)
Q, S, T, DD = 100, 25, 128, 2048
NCORES = 8
QC = 13                 # queries per core; Q padded to 104
QPAD = QC * NCORES
NKK = DD // 256         # 8 fp8 DoubleRow contraction tiles (256 deep each)
SJ = S * T              # 3200 = flattened (support, j)
B = QC * S              # 325 pairs per core
# DP pair-tile streams aligned to query boundaries (offset, count):
# a stream's first row can run as soon as its LAST query's cost matrix is
# in DRAM. 3/5/5 queries: the DVE wavefront starts at q2 instead of q4.
PT = [(0, 75), (75, 125), (200, 125)]   # q0-2 | q3-7 | q8-12
# stage-A processing order of the 13 query slots. Streams gate on their
# LAST slot: s1 on slot 2, s2 on slot 7, s3 on slot 12.
QORDER = list(range(13))
# DP row-windows per gather DMA: small first window so each stream's
# wavefront starts ~2us earlier (less gather gen+transfer to wait on).
# Stream 2 (middle) uses smaller windows + a single-buffered gather pool:
# its rows are RELEASED gradually, so the greedy scheduler cannot drain
# stream 2 early and leave stream 3 alone (at 66% serial rate) in the tail.
def _wins(first, mid):
    w, i = [(0, first)], first
    while i < T:
        n = min(mid, T - i)
        w.append((i, n))
        i += n
    return w[:len(w)]
WINS_S = {0: _wins(4, 16), 1: _wins(4, 16), 2: _wins(4, 16)}
for _w in WINS_S.values():
    assert sum(n for _, n in _w) == T
CH = 512                # matmul moving-chunk / PSUM bank width
# all chunks >=256 so the fp32r rank-2 matmul stays at 1 cycle/row
_CW = [512, 512, 512, 512, 512, 384, 256]
CHUNKS = [(sum(_CW[:i]), w) for i, w in enumerate(_CW)]
assert sum(_CW) == SJ
BIG = 1e10

_built = None          # cached compiled Bass program
_last_result = None    # last BassKernelResults (exec_time_ns when traced)
_predicted_ns = None   # Tile cost-model makespan of the per-core program


def _build():
    import concourse.bacc as bacc
    import concourse.mybir as mybir
    import concourse.tile as tile

    f32 = mybir.dt.float32
    f32r = mybir.dt.float32r
    bf16 = mybir.dt.bfloat16
    fp8 = mybir.dt.float8e4
    MIN = mybir.AluOpType.min
    ADD = mybir.AluOpType.add
    DR = mybir.MatmulPerfMode.DoubleRow

    global _predicted_ns
    nc = bacc.Bacc("TRN2", debug=False)

    xt_d = nc.dram_tensor("xt", [QC, 128, NKK, 2, T], fp8, kind="ExternalInput")
    yt_d = nc.dram_tensor("yt", [128, NKK, 2, SJ], fp8, kind="ExternalInput")
    # aux DoubleRow rows (K_p=2, 2 slabs = 4 contraction rows):
    #   (k0,s0): 64 * 64            -> +4096 (exact mean of x2+y2)
    #   (k1,s0): rx[i]/8 * 8        -> +(x2[i]-2048) residual
    #   (k0,s1): 8 * ry[sj]/8       -> +(y2[sj]-2048) residual
    #   (k1,s1): 0
    xa_d = nc.dram_tensor("xa", [QC, 2, 2, T], fp8, kind="ExternalInput")
    ya_d = nc.dram_tensor("ya", [2, 2, SJ], fp8, kind="ExternalInput")
    out_d = nc.dram_tensor("out_cd", [QC, S], f32, kind="ExternalOutput")
    # cost matrices staged pair-major and ZERO-INTERLEAVED: [q, s, i, 2T]
    # holds (0, d) pairs so the DP can run ONE fused scan per row:
    #   step even: state = min(R_prev[c], state) + 0
    #   step odd:  state = min(R_prev[c+1], state) + d_c   (= R_cur[c])
    dsc = nc.dram_tensor("dsc", [QC, S, T, 2 * T], bf16)
    dsc_p = dsc[:].rearrange("q s i j2 -> (q s) i j2")

    with tile.TileContext(nc) as tc:
        with (
            tc.tile_pool(name="const", bufs=1) as constp,
            tc.tile_pool(name="xq", bufs=2) as xqp,
            tc.tile_pool(name="augq", bufs=2) as augqp,
            tc.tile_pool(name="psum", bufs=8, space="PSUM") as psump,
            tc.tile_pool(name="dq", bufs=1) as dqp,
            tc.tile_pool(name="ga", bufs=2) as gap,
            tc.tile_pool(name="gb", bufs=1) as gbp,
            tc.tile_pool(name="gc", bufs=2) as gcp,
            tc.tile_pool(name="dp", bufs=1) as dpp,
        ):
            # q0's operands first on the ACT queue (ahead of the yt pieces).
            xt0_sb = xqp.tile([128, NKK, 2, T], fp8, tag="xt")
            nc.scalar.dma_start(xt0_sb[:], xt_d[QORDER[0]])
            xa0_sb = augqp.tile([2, 2, T], fp8, tag="xa")
            nc.scalar.dma_start(xa0_sb[:], xa_d[QORDER[0]])

            # Resident Y^T (fp8), per K-tile so q0 starts after ~800KB,
            # spread across 3 DMA queues (SP + ACT + Pool/SWDGE) so all of
            # yt lands within ~7us. The Pool sequencer is idle until the
            # wavefront starts, so the SWDGE generation there is free.
            yt_sb = constp.tile([128, NKK, 2, SJ], fp8)
            for k in range(NKK):
                qeng = [nc.sync, nc.scalar, nc.gpsimd][k % 3]
                qeng.dma_start(yt_sb[:, k, :, :], yt_d[:, k, :, :])
            ya_sb = constp.tile([2, 2, SJ], fp8)
            nc.sync.dma_start(ya_sb[:], ya_d[:])

            # Two persistent interleaved staging buffers [128, 2*SJ]:
            # odd slots take the PSUM evacuation (strided ACT writes), even
            # slots are zeroed ONCE here (idle DVE) and persist physically.
            dq_bufs = []
            for di in range(2):
                dq_t = dqp.tile([128, 2 * SJ], bf16, tag=f"dq{di}")
                dq_bufs.append(dq_t)
                dq_ev = dq_t[:].rearrange("p (sj two) -> p two sj", two=2)
                for c0, cw in CHUNKS:
                    nc.vector.memset(dq_ev[:, 0, c0:c0 + cw], 0.0)

            # ---- Stage A: cost matrices, one query at a time (k-outer) ----
            xt_tiles = {QORDER[0]: xt0_sb}
            xa_tiles = {QORDER[0]: xa0_sb}
            for qi, q in enumerate(QORDER):
                xt_sb, xa_sb = xt_tiles.pop(q), xa_tiles.pop(q)
                dq_sb = dq_bufs[qi % 2]
                dq_odd = dq_sb[:].rearrange("p (sj two) -> p two sj", two=2)
                # k-OUTER: each fp8 weight tile streams all 7 chunks (7 PSUM
                # banks live), so weight loads amortize 7x; for q0 every
                # arriving yt K-tile immediately feeds all chunks, so q0's
                # matrix completes with the prologue.
                pss = []
                for _ci in range(len(CHUNKS)):
                    ps_t = psump.tile([128, CH], f32, tag="ps")
                    pss.append(ps_t)
                for kk in range(NKK):
                    for ci, (c0, cw) in enumerate(CHUNKS):
                        nc.tensor.matmul(
                            pss[ci][:, :cw],
                            xt_sb[:, kk, :, :],
                            yt_sb[:, kk, :, c0:c0 + cw],
                            start=(kk == 0),
                            stop=False,
                            perf_mode=DR,
                        )
                for ci, (c0, cw) in enumerate(CHUNKS):
                    # fp8 aux rank-4: + 4096 + rx[i] + ry[s,j]
                    nc.tensor.matmul(
                        pss[ci][:, :cw],
                        xa_sb[:, :, :],
                        ya_sb[:, :, c0:c0 + cw],
                        start=False,
                        stop=True,
                        perf_mode=DR,
                    )
                    nc.scalar.copy(dq_odd[:, 1, c0:c0 + cw], pss[ci][:, :cw])
                    if ci == 0 and qi + 1 < QC:
                        # next query's operands on the SP ring, emitted ahead
                        # of this query's dsc writes: keeps the ACT queue a
                        # pure evac chain (the gather latency of gate queries
                        # depends on it) and still lands with ~5 chunks of
                        # margin before q+1's first matmul.
                        qn = QORDER[qi + 1]
                        xt_sb_n = xqp.tile([128, NKK, 2, T], fp8, tag="xt")
                        nc.sync.dma_start(xt_sb_n[:], xt_d[qn])
                        xa_sb_n = augqp.tile([2, 2, T], fp8, tag="xa")
                        nc.sync.dma_start(xa_sb_n[:], xa_d[qn])
                        xt_tiles[qn] = xt_sb_n
                        xa_tiles[qn] = xa_sb_n
                    # per-chunk dsc write (chunk widths are whole s-blocks)
                    nc.sync.dma_start(
                        dsc[q, c0 // T:(c0 + cw) // T]
                        .rearrange("s i j2 -> i s j2"),
                        dq_sb[:, 2 * c0:2 * (c0 + cw)]
                        .rearrange("i (s j2) -> i s j2", j2=2 * T))

            # ---- Stage B: hard-DTW wavefront, 3 batched pair-tiles ----
            # One FUSED scan per row (free = 2T): the 3-way min is expanded
            # into two scan steps per cell, data0 = overlapping (R[c], R[c+1])
            # pairs of the interleaved prev rowbuf (hand-built strided AP),
            # data1 = (0, d) pairs from the interleaved gather tiles. No
            # separate mud instruction, one fewer dep link per row.
            out_flat = out_d[:].rearrange("q s -> (q s)")
            T2 = 2 * T

            def fused_row_scan(cur_ap, prev_tile, np_, g_ap):
                # data0: [np, 1+2T) viewed as [np, T, 2] with strides (2, 2)
                a = prev_tile[:np_, 1:T2 + 1].copy()
                ap0 = [list(x) for x in a.ap]
                assert ap0[-1][0] == 1 and ap0[-1][1] == T2, ap0
                a.ap = mybir.VecI64Pair(ap0[:-1] + [[2, T], [2, 2]])
                nc.vector.add_instruction(
                    mybir.InstTensorScalarPtr(
                        name=nc.get_next_instruction_name(),
                        is_tensor_tensor_scan=True,
                        is_scalar_tensor_tensor=True,
                        op0=MIN,
                        op1=ADD,
                        ins=[nc.vector.lower_ap(a),
                             mybir.ImmediateValue(dtype=f32, value=BIG),
                             nc.vector.lower_ap(g_ap)],
                        outs=[nc.vector.lower_ap(cur_ap)],
                    )
                )

            for pt, (p0, np_) in enumerate(PT):
                gpool = [gap, gbp, gcp][pt]

                # interleaved rowbufs: [pad, border, (partial, R) * T, pad]
                r_a = dpp.tile([128, T2 + 4], f32, tag=f"ra{pt}")
                r_b = dpp.tile([128, T2 + 4], f32, tag=f"rb{pt}")
                # row 0 of the R-grid lives in r_a's odd slots: corner 0 at
                # the border slot 1, BIG elsewhere; r_b border = BIG.
                # memsets on Pool: keeps them off the DVE critical chain.
                nc.gpsimd.memset(r_a[:np_, 1:T2 + 2], BIG)
                nc.gpsimd.memset(r_a[:np_, 1:2], 0.0)
                nc.gpsimd.memset(r_b[:np_, 1:2], BIG)

                WINS = WINS_S[pt]
                win_of = {}
                for wi, (w0, wn) in enumerate(WINS):
                    for i in range(w0, w0 + wn):
                        win_of[i] = (wi, w0)
                g_tiles = {}
                for i in range(T):
                    wi, w0 = win_of[i]
                    if i == w0:
                        wn = WINS[wi][1]
                        g_t = gpool.tile([128, 16 * T2], bf16, tag=f"g{pt}")
                        g_tiles[wi] = g_t
                        # One DMA per window (full SDMA-engine spread).
                        # Pool/SWDGE: idle sequencer, not paced by ACT/SP.
                        # Window 0 splits off the final query so the earlier
                        # queries prefetch while the last matrix is still
                        # being written.
                        if pt >= 1 and wi == 0:
                            cut = np_ - S
                            nc.gpsimd.dma_start(
                                g_t[:cut, :wn * T2].rearrange(
                                    "p (w j2) -> p w j2", j2=T2),
                                dsc_p[p0:p0 + cut, w0:w0 + wn, :],
                            )
                            nc.gpsimd.dma_start(
                                g_t[cut:np_, :wn * T2].rearrange(
                                    "p (w j2) -> p w j2", j2=T2),
                                dsc_p[p0 + cut:p0 + np_, w0:w0 + wn, :],
                            )
                        else:
                            nc.gpsimd.dma_start(
                                g_t[:np_, :wn * T2].rearrange(
                                    "p (w j2) -> p w j2", j2=T2),
                                dsc_p[p0:p0 + np_, w0:w0 + wn, :],
                            )
                    g_t = g_tiles[wi]
                    prev, cur = (r_a, r_b) if i % 2 == 0 else (r_b, r_a)
                    fused_row_scan(
                        cur[:np_, 2:T2 + 2], prev, np_,
                        g_t[:np_, (i - w0) * T2:(i - w0 + 1) * T2])
                    if i == 0:
                        # row-0 buffer becomes an interior row: border 0 -> BIG
                        nc.vector.memset(prev[:np_, 1:2], BIG)

                final = r_b if T % 2 == 1 else r_a  # T=128 even -> last cur=r_a
                nc.sync.dma_start(
                    out_flat[p0:p0 + np_], final[:np_, T2 + 1:T2 + 2])

    ents = getattr(tc, "_perfetto_entries", None)
    if ents:
        _predicted_ns = int(max(e[2] for e in ents))
    nc.compile()
    return nc


def _pack_inputs(X, Yf):
    """Host-side packing into the exact SBUF layouts the kernel DMAs 1:1."""
    f8 = ml_dtypes.float8_e4m3
    # xt: [QPAD, 128(dk), NKK, 2, T] = fp8(-2*X)^T, DoubleRow slab layout
    # contraction index d = kk*256 + slab*128 + dk
    Xp = np.zeros((QPAD, T, DD), np.float32)
    Xp[:Q] = X
    xtq = np.ascontiguousarray(
        (-2.0 * Xp).astype(f8).transpose(0, 2, 1)        # [QPAD, DD, T]
        .reshape(QPAD, NKK, 2, 128, T).transpose(0, 3, 1, 2, 4))
    # yt: [128(dk), NKK, 2, SJ] = fp8(Y)^T
    yt = np.ascontiguousarray(
        Yf.astype(f8).transpose(2, 0, 1)                 # [DD, S, T]
        .reshape(NKK, 2, 128, SJ).transpose(2, 0, 1, 3))
    # norms folded into fp8 aux DoubleRow rows:
    # x2+y2 = 4096 (exact, 64*64) + rx + ry residuals (fp8/8, err ~+-8)
    x2 = np.einsum("qtd,qtd->qt", Xp, Xp, dtype=np.float32)  # [QPAD, T]
    y2 = np.einsum("std,std->st", Yf, Yf, dtype=np.float32)  # [S, T]
    xa = np.zeros((QPAD, 2, 2, T), f8)
    xa[:, 0, 0, :] = f8(64.0)
    xa[:, 1, 0, :] = ((x2 - 2048.0) / 8.0).astype(f8)
    xa[:, 0, 1, :] = f8(8.0)
    ya = np.zeros((2, 2, SJ), f8)
    ya[0, 0, :] = f8(64.0)
    ya[1, 0, :] = f8(8.0)
    ya[0, 1, :] = ((y2.reshape(SJ) - 2048.0) / 8.0).astype(f8)
    return xtq, yt, xa, ya


def kernel(support_features, support_labels, target_features, n_classes):
    global _built
    from concourse.bass_utils import run_bass_kernel_spmd

    X = np.asarray(target_features, dtype=np.float32)
    Yf = np.asarray(support_features, dtype=np.float32)
    labels = np.asarray(support_labels)
    ncls = int(np.asarray(n_classes))
    assert X.shape == (Q, T, DD) and Yf.shape == (S, T, DD), (
        f"kernel compiled for fixed shapes; got {X.shape}, {Yf.shape}")

    xtq, yt, xa, ya = _pack_inputs(X, Yf)

    if _built is None:
        _built = _build()
    nc = _built

    in_maps = [
        {
            "xt": np.ascontiguousarray(xtq[c * QC:(c + 1) * QC]),
            "yt": yt,
            "xa": np.ascontiguousarray(xa[c * QC:(c + 1) * QC]),
            "ya": ya,
        }
        for c in range(NCORES)
    ]
    res = run_bass_kernel_spmd(nc, in_maps, list(range(NCORES)))
    global _last_result
    _last_result = res
    cum = np.concatenate([res.results[c]["out_cd"] for c in range(NCORES)])[:Q]

    onehot = (labels[:, None] == np.arange(ncls)[None, :]).astype(np.float32)
    counts = np.maximum(onehot.sum(axis=0), 1.0).astype(np.float32)
    logits = -(cum.astype(np.float32) @ onehot) / counts
    return logits.astype(np.float32)


# revision 5
# speedup vs baseline: 2.1800x; 1.0681x over previous
"""Trainium2 Bass kernel: CNN-feature SoftDTW few-shot classifier (v2).

Computes, for Q=100 query sequences and S=25 support sequences (T=128 steps,
D=2048 features): pairwise squared-euclidean cost matrices, soft-DTW alignment
cost per (query, support) pair, then per-class mean distances -> logits.

Key numerical fact: with gamma=0.1 and cost magnitudes ~4096, the reference's
fp32 softmin is bitwise the hard min (exp((m-x)/gamma) underflows for every
non-minimal branch), so the DP is computed with min/add only. Each DP row is
one `tensor_tensor_scan(op0=min, op1=add)` instruction.

v3 changes vs the 385us baseline:
  - fp8e4m3 DoubleRow matmul (0.5 cy/row, K=256/instr) for xy: PE work for
    the 2048-deep contraction drops 4x vs bf16. Quantization error on the
    DTW path sum is ~1e-4 of the output scale (threshold 2e-2).
  - k-outer matmul order for EVERY query (7 PSUM banks live) so each fp8
    weight tile is loaded once per 3200-column sweep.
  - cost matrices staged in DRAM as bf16 (+-8 absolute per ~4096-cell):
    halves the dsc round-trip DMA. DVE scan reads bf16 data1 directly.
  - DP pair streams split 3/5/5 queries so the wavefront starts after only
    3 query matrices instead of 5.

Sharding: data-parallel over queries, 13 per core (Q padded 100->104),
supports replicated. Host: fp8 packing/transposes, x2/y2 sums, final
class-mean logits.
"""

import sys

for _p in ("/opt/trn_rl_repo",):
    if _p not in sys.path:
        sys.path.insert(0, _p)

import numpy as np
import ml_dtypes

# Problem shape (hardcoded: harness runs kernel.py standalone)
Q, S, T, DD = 100, 25, 128, 2048
NCORES = 8
QC = 13                 # queries per core; Q padded to 104
QPAD = QC * NCORES
NKK = DD // 256         # 8 fp8 DoubleRow contraction tiles (256 deep each)
SJ = S * T              # 3200 = flattened (support, j)
B = QC * S              # 325 pairs per core
# DP pair-tile streams aligned to query boundaries (offset, count):
# a stream's first row can run as soon as its LAST query's cost matrix is
# in DRAM. 3/5/5 queries: the DVE wavefront starts at q2 instead of q4.
PT = [(0, 75), (75, 125), (200, 125)]   # q0-2 | q3-7 | q8-12
# stage-A processing order of the 13 query slots. Streams gate on their
# LAST slot: s1 on slot 2, s2 on slot 7, s3 on slot 12.
QORDER = list(range(13))
# DP row-windows per gather DMA: small first window so each stream's
# wavefront starts ~2us earlier (less gather gen+transfer to wait on).
# Stream 2 (middle) uses smaller windows + a single-buffered gather pool:
# its rows are RELEASED gradually, so the greedy scheduler cannot drain
# stream 2 early and leave stream 3 alone (at 66% serial rate) in the tail.
def _wins(first, mid):
    w, i = [(0, first)], first
    while i < T:
        n = min(mid, T - i)
        w.append((i, n))
        i += n
    return w[:len(w)]
WINS_S = {0: _wins(4, 16), 1: _wins(4, 16), 2: _wins(4, 16)}
for _w in WINS_S.values():
    assert sum(n for _, n in _w) == T
CH = 512                # matmul moving-chunk / PSUM bank width
# all chunks >=256 so the fp32r rank-2 matmul stays at 1 cycle/row
_CW = [512, 512, 512, 512, 512, 384, 256]
CHUNKS = [(sum(_CW[:i]), w) for i, w in enumerate(_CW)]
assert sum(_CW) == SJ
BIG = 1e10

_built = None          # cached compiled Bass program
_last_result = None    # last BassKernelResults (exec_time_ns when traced)
_predicted_ns = None   # Tile cost-model makespan of the per-core program


def _build():
    import concourse.bacc as bacc
    import concourse.mybir as mybir
    import concourse.tile as tile

    f32 = mybir.dt.float32
    f32r = mybir.dt.float32r
    bf16 = mybir.dt.bfloat16
    fp8 = mybir.dt.float8e4
    MIN = mybir.AluOpType.min
    ADD = mybir.AluOpType.add
    DR = mybir.MatmulPerfMode.DoubleRow

    global _predicted_ns
    nc = bacc.Bacc("TRN2", debug=False)

    xt_d = nc.dram_tensor("xt", [QC, 128, NKK, 2, T], fp8, kind="ExternalInput")
    yt_d = nc.dram_tensor("yt", [128, NKK, 2, SJ], fp8, kind="ExternalInput")
    # aux DoubleRow rows (K_p=2, 2 slabs = 4 contraction rows):
    #   (k0,s0): 64 * 64            -> +4096 (exact mean of x2+y2)
    #   (k1,s0): rx[i]/8 * 8        -> +(x2[i]-2048) residual
    #   (k0,s1): 8 * ry[sj]/8       -> +(y2[sj]-2048) residual
    #   (k1,s1): 0
    xa_d = nc.dram_tensor("xa", [QC, 2, 2, T], fp8, kind="ExternalInput")
    ya_d = nc.dram_tensor("ya", [2, 2, SJ], fp8, kind="ExternalInput")
    out_d = nc.dram_tensor("out_cd", [QC, S], f32, kind="ExternalOutput")
    # cost matrices staged pair-major and ZERO-INTERLEAVED: [q, s, i, 2T]
    # holds (0, d) pairs so the DP can run ONE fused scan per row:
    #   step even: state = min(R_prev[c], state) + 0
    #   step odd:  state = min(R_prev[c+1], state) + d_c   (= R_cur[c])
    dsc = nc.dram_tensor("dsc", [QC, S, T, 2 * T], bf16)
    dsc_p = dsc[:].rearrange("q s i j2 -> (q s) i j2")

    with tile.TileContext(nc) as tc:
        with (
            tc.tile_pool(name="const", bufs=1) as constp,
            tc.tile_pool(name="xq", bufs=2) as xqp,
            tc.tile_pool(name="augq", bufs=2) as augqp,
            tc.tile_pool(name="psum", bufs=8, space="PSUM") as psump,
            tc.tile_pool(name="dq", bufs=1) as dqp,
            tc.tile_pool(name="ga", bufs=2) as gap,
            tc.tile_pool(name="gb", bufs=2) as gbp,
            tc.tile_pool(name="gc", bufs=2) as gcp,
            tc.tile_pool(name="dp", bufs=1) as dpp,
        ):
            # q0's operands first on the ACT queue (ahead of the yt pieces).
            xt0_sb = xqp.tile([128, NKK, 2, T], fp8, tag="xt")
            nc.scalar.dma_start(xt0_sb[:], xt_d[QORDER[0]])
            xa0_sb = augqp.tile([2, 2, T], fp8, tag="xa")
            nc.scalar.dma_start(xa0_sb[:], xa_d[QORDER[0]])

            # Resident Y^T (fp8), per K-tile so q0 starts after ~800KB,
            # spread across 3 DMA queues (SP + ACT + Pool/SWDGE) so all of
            # yt lands within ~7us. The Pool sequencer is idle until the
            # wavefront starts, so the SWDGE generation there is free.
            yt_sb = constp.tile([128, NKK, 2, SJ], fp8)
            for k in range(NKK):
                qeng = [nc.sync, nc.scalar, nc.gpsimd][k % 3]
                qeng.dma_start(yt_sb[:, k, :, :], yt_d[:, k, :, :])
            ya_sb = constp.tile([2, 2, SJ], fp8)
            nc.sync.dma_start(ya_sb[:], ya_d[:])

            # Two persistent interleaved staging buffers [128, 2*SJ]:
            # odd slots take the PSUM evacuation (strided ACT writes), even
            # slots are zeroed ONCE here (idle DVE) and persist physically.
            dq_bufs = []
            for di in range(2):
                dq_t = dqp.tile([128, 2 * SJ], bf16, tag=f"dq{di}")
                dq_bufs.append(dq_t)
                dq_ev = dq_t[:].rearrange("p (sj two) -> p two sj", two=2)
                for c0, cw in CHUNKS:
                    nc.vector.memset(dq_ev[:, 0, c0:c0 + cw], 0.0)

            # ---- Stage A: cost matrices, one query at a time (k-outer) ----
            xt_tiles = {QORDER[0]: xt0_sb}
            xa_tiles = {QORDER[0]: xa0_sb}
            for qi, q in enumerate(QORDER):
                xt_sb, xa_sb = xt_tiles.pop(q), xa_tiles.pop(q)
                dq_sb = dq_bufs[qi % 2]
                dq_odd = dq_sb[:].rearrange("p (sj two) -> p two sj", two=2)
                # k-OUTER: each fp8 weight tile streams all 7 chunks (7 PSUM
                # banks live), so weight loads amortize 7x; for q0 every
                # arriving yt K-tile immediately feeds all chunks, so q0's
                # matrix completes with the prologue.
                pss = []
                for _ci in range(len(CHUNKS)):
                    ps_t = psump.tile([128, CH], f32, tag="ps")
                    pss.append(ps_t)
                for kk in range(NKK):
                    for ci, (c0, cw) in enumerate(CHUNKS):
                        nc.tensor.matmul(
                            pss[ci][:, :cw],
                            xt_sb[:, kk, :, :],
                            yt_sb[:, kk, :, c0:c0 + cw],
                            start=(kk == 0),
                            stop=False,
                            perf_mode=DR,
                        )
                for ci, (c0, cw) in enumerate(CHUNKS):
                    # fp8 aux rank-4: + 4096 + rx[i] + ry[s,j]
                    nc.tensor.matmul(
                        pss[ci][:, :cw],
                        xa_sb[:, :, :],
                        ya_sb[:, :, c0:c0 + cw],
                        start=False,
                        stop=True,
                        perf_mode=DR,
                    )
                    nc.scalar.copy(dq_odd[:, 1, c0:c0 + cw], pss[ci][:, :cw])
                    if ci == 0 and qi + 1 < QC:
                        # next query's operands on the SP ring, emitted ahead
                        # of this query's dsc writes: keeps the ACT queue a
                        # pure evac chain (the gather latency of gate queries
                        # depends on it) and still lands with ~5 chunks of
                        # margin before q+1's first matmul.
                        qn = QORDER[qi + 1]
                        xt_sb_n = xqp.tile([128, NKK, 2, T], fp8, tag="xt")
                        nc.sync.dma_start(xt_sb_n[:], xt_d[qn])
                        xa_sb_n = augqp.tile([2, 2, T], fp8, tag="xa")
                        nc.sync.dma_start(xa_sb_n[:], xa_d[qn])
                        xt_tiles[qn] = xt_sb_n
                        xa_tiles[qn] = xa_sb_n
                    # per-chunk dsc write (chunk widths are whole s-blocks)
                    nc.sync.dma_start(
                        dsc[q, c0 // T:(c0 + cw) // T]
                        .rearrange("s i j2 -> i s j2"),
                        dq_sb[:, 2 * c0:2 * (c0 + cw)]
                        .rearrange("i (s j2) -> i s j2", j2=2 * T))

            # ---- Stage B: hard-DTW wavefront, 3 batched pair-tiles ----
            # One FUSED scan per row (free = 2T): the 3-way min is expanded
            # into two scan steps per cell, data0 = overlapping (R[c], R[c+1])
            # pairs of the interleaved prev rowbuf (hand-built strided AP),
            # data1 = (0, d) pairs from the interleaved gather tiles. No
            # separate mud instruction, one fewer dep link per row.
            out_flat = out_d[:].rearrange("q s -> (q s)")
            T2 = 2 * T

            def fused_row_scan(cur_ap, prev_tile, np_, g_ap):
                # data0: [np, 1+2T) viewed as [np, T, 2] with strides (2, 2)
                a = prev_tile[:np_, 1:T2 + 1].copy()
                ap0 = [list(x) for x in a.ap]
                assert ap0[-1][0] == 1 and ap0[-1][1] == T2, ap0
                a.ap = mybir.VecI64Pair(ap0[:-1] + [[2, T], [2, 2]])
                nc.vector.add_instruction(
                    mybir.InstTensorScalarPtr(
                        name=nc.get_next_instruction_name(),
                        is_tensor_tensor_scan=True,
                        is_scalar_tensor_tensor=True,
                        op0=MIN,
                        op1=ADD,
                        ins=[nc.vector.lower_ap(a),
                             mybir.ImmediateValue(dtype=f32, value=BIG),
                             nc.vector.lower_ap(g_ap)],
                        outs=[nc.vector.lower_ap(cur_ap)],
                    )
                )

            for pt, (p0, np_) in enumerate(PT):
                gpool = [gap, gbp, gcp][pt]

                # interleaved rowbufs: [pad, border, (partial, R) * T, pad]
                r_a = dpp.tile([128, T2 + 4], f32, tag=f"ra{pt}")
                r_b = dpp.tile([128, T2 + 4], f32, tag=f"rb{pt}")
                # row 0 of the R-grid lives in r_a's odd slots: corner 0 at
                # the border slot 1, BIG elsewhere; r_b border = BIG.
                # memsets on Pool: keeps them off the DVE critical chain.
                nc.gpsimd.memset(r_a[:np_, 1:T2 + 2], BIG)
                nc.gpsimd.memset(r_a[:np_, 1:2], 0.0)
                nc.gpsimd.memset(r_b[:np_, 1:2], BIG)

                WINS = WINS_S[pt]
                win_of = {}
                for wi, (w0, wn) in enumerate(WINS):
                    for i in range(w0, w0 + wn):
                        win_of[i] = (wi, w0)
                g_tiles = {}
                for i in range(T):
                    wi, w0 = win_of[i]
                    if i == w0:
                        wn = WINS[wi][1]
                        g_t = gpool.tile([128, 16 * T2], bf16, tag=f"g{pt}")
                        g_tiles[wi] = g_t
                        # One DMA per window (full SDMA-engine spread).
                        # Pool/SWDGE: idle sequencer, not paced by ACT/SP.
                        # Window 0 splits off the final query so the earlier
                        # queries prefetch while the last matrix is still
                        # being written.
                        if pt >= 1 and wi == 0:
                            cut = np_ - S
                            nc.gpsimd.dma_start(
                                g_t[:cut, :wn * T2].rearrange(
                                    "p (w j2) -> p w j2", j2=T2),
                                dsc_p[p0:p0 + cut, w0:w0 + wn, :],
                            )
                            nc.gpsimd.dma_start(
                                g_t[cut:np_, :wn * T2].rearrange(
                                    "p (w j2) -> p w j2", j2=T2),
                                dsc_p[p0 + cut:p0 + np_, w0:w0 + wn, :],
                            )
                        else:
                            nc.gpsimd.dma_start(
                                g_t[:np_, :wn * T2].rearrange(
                                    "p (w j2) -> p w j2", j2=T2),
                                dsc_p[p0:p0 + np_, w0:w0 + wn, :],
                            )
                    g_t = g_tiles[wi]
                    prev, cur = (r_a, r_b) if i % 2 == 0 else (r_b, r_a)
                    fused_row_scan(
                        cur[:np_, 2:T2 + 2], prev, np_,
                        g_t[:np_, (i - w0) * T2:(i - w0 + 1) * T2])
                    if i == 0:
                        # row-0 buffer becomes an interior row: border 0 -> BIG
                        nc.vector.memset(prev[:np_, 1:2], BIG)

                final = r_b if T % 2 == 1 else r_a  # T=128 even -> last cur=r_a
                nc.sync.dma_start(
                    out_flat[p0:p0 + np_], final[:np_, T2 + 1:T2 + 2])

    ents = getattr(tc, "_perfetto_entries", None)
    if ents:
        _predicted_ns = int(max(e[2] for e in ents))
    nc.compile()
    return nc


def _pack_inputs(X, Yf):
    """Host-side packing into the exact SBUF layouts the kernel DMAs 1:1."""
    f8 = ml_dtypes.float8_e4m3
    # xt: [QPAD, 128(dk), NKK, 2, T] = fp8(-2*X)^T, DoubleRow slab layout
    # contraction index d = kk*256 + slab*128 + dk
    Xp = np.zeros((QPAD, T, DD), np.float32)
    Xp[:Q] = X
    xtq = np.ascontiguousarray(
        (-2.0 * Xp).astype(f8).transpose(0, 2, 1)        # [QPAD, DD, T]
        .reshape(QPAD, NKK, 2, 128, T).transpose(0, 3, 1, 2, 4))
    # yt: [128(dk), NKK, 2, SJ] = fp8(Y)^T
    yt = np.ascontiguousarray(
        Yf.astype(f8).transpose(2, 0, 1)                 # [DD, S, T]
        .reshape(NKK, 2, 128, SJ).transpose(2, 0, 1, 3))
    # norms folded into fp8 aux DoubleRow rows:
    # x2+y2 = 4096 (exact, 64*64) + rx + ry residuals (fp8/8, err ~+-8)
    x2 = np.einsum("qtd,qtd->qt", Xp, Xp, dtype=np.float32)  # [QPAD, T]
    y2 = np.einsum("std,std->st", Yf, Yf, dtype=np.float32)  # [S, T]
    xa = np.zeros((QPAD, 2, 2, T), f8)
    xa[:, 0, 0, :] = f8(64.0)
    xa[:, 1, 0, :] = ((x2 - 2048.0) / 8.0).astype(f8)
    xa[:, 0, 1, :] = f8(8.0)
    ya = np.zeros((2, 2, SJ), f8)
    ya[0, 0, :] = f8(64.0)
    ya[1, 0, :] = f8(8.0)
    ya[0, 1, :] = ((y2.reshape(SJ) - 2048.0) / 8.0).astype(f8)
    return xtq, yt, xa, ya


def kernel(support_features, support_labels, target_features, n_classes):
    global _built
    from concourse.bass_utils import run_bass_kernel_spmd

    X = np.asarray(target_features, dtype=np.float32)
    Yf = np.asarray(support_features, dtype=np.float32)
    labels = np.asarray(support_labels)
    ncls = int(np.asarray(n_classes))
    assert X.shape == (Q, T, DD) and Yf.shape == (S, T, DD), (
        f"kernel compiled for fixed shapes; got {X.shape}, {Yf.shape}")

    xtq, yt, xa, ya = _pack_inputs(X, Yf)

    if _built is None:
        _built = _build()
    nc = _built

    in_maps = [
        {
            "xt": np.ascontiguousarray(xtq[c * QC:(c + 1) * QC]),
            "yt": yt,
            "xa": np.ascontiguousarray(xa[c * QC:(c + 1) * QC]),
            "ya": ya,
        }
        for c in range(NCORES)
    ]
    res = run_bass_kernel_spmd(nc, in_maps, list(range(NCORES)))
    global _last_result
    _last_result = res
    cum = np.concatenate([res.results[c]["out_cd"] for c in range(NCORES)])[:Q]

    onehot = (labels[:, None] == np.arange(ncls)[None, :]).astype(np.float32)
    counts = np.maximum(onehot.sum(axis=0), 1.0).astype(np.float32)
    logits = -(cum.astype(np.float32) @ onehot) / counts
    return logits.astype(np.float32)


# revision 6
# speedup vs baseline: 2.2153x; 1.0162x over previous
"""Trainium2 Bass kernel: CNN-feature SoftDTW few-shot classifier (v2).

Computes, for Q=100 query sequences and S=25 support sequences (T=128 steps,
D=2048 features): pairwise squared-euclidean cost matrices, soft-DTW alignment
cost per (query, support) pair, then per-class mean distances -> logits.

Key numerical fact: with gamma=0.1 and cost magnitudes ~4096, the reference's
fp32 softmin is bitwise the hard min (exp((m-x)/gamma) underflows for every
non-minimal branch), so the DP is computed with min/add only. Each DP row is
one `tensor_tensor_scan(op0=min, op1=add)` instruction.

v3 changes vs the 385us baseline:
  - fp8e4m3 DoubleRow matmul (0.5 cy/row, K=256/instr) for xy: PE work for
    the 2048-deep contraction drops 4x vs bf16. Quantization error on the
    DTW path sum is ~1e-4 of the output scale (threshold 2e-2).
  - k-outer matmul order for EVERY query (7 PSUM banks live) so each fp8
    weight tile is loaded once per 3200-column sweep.
  - cost matrices staged in DRAM as bf16 (+-8 absolute per ~4096-cell):
    halves the dsc round-trip DMA. DVE scan reads bf16 data1 directly.
  - DP pair streams split 3/5/5 queries so the wavefront starts after only
    3 query matrices instead of 5.

Sharding: data-parallel over queries, 13 per core (Q padded 100->104),
supports replicated. Host: fp8 packing/transposes, x2/y2 sums, final
class-mean logits.
"""

import sys

for _p in ("/opt/trn_rl_repo",):
    if _p not in sys.path:
        sys.path.insert(0, _p)

import numpy as np
import ml_dtypes

# Problem shape (hardcoded: harness runs kernel.py standalone)
Q, S, T, DD = 100, 25, 128, 2048
NCORES = 8
QC = 13                 # queries per core; Q padded to 104
QPAD = QC * NCORES
NKK = DD // 256         # 8 fp8 DoubleRow contraction tiles (256 deep each)
SJ = S * T              # 3200 = flattened (support, j)
B = QC * S              # 325 pairs per core
# DP pair-tile streams aligned to query boundaries (offset, count):
# a stream's first row can run as soon as its LAST query's cost matrix is
# in DRAM. 3/5/5 queries: the DVE wavefront starts at q2 instead of q4.
PT = [(0, 75), (75, 125), (200, 125)]   # q0-2 | q3-7 | q8-12
# stage-A processing order of the 13 query slots. Streams gate on their
# LAST slot: s1 on slot 2, s2 on slot 7, s3 on slot 12.
# Process s3's queries (slots 8-12) right after s1's gate: the wavefront
# stream gates land at positions (3, 13, 8) of 13 - the two big streams
# start as early as possible and the middle one last.
QORDER = [0, 1, 2, 8, 9, 10, 11, 12, 3, 4, 5, 6, 7]
# DP row-windows per gather DMA: small first window so each stream's
# wavefront starts ~2us earlier (less gather gen+transfer to wait on).
# Stream 2 (middle) uses smaller windows + a single-buffered gather pool:
# its rows are RELEASED gradually, so the greedy scheduler cannot drain
# stream 2 early and leave stream 3 alone (at 66% serial rate) in the tail.
def _wins(first, mid):
    w, i = [(0, first)], first
    while i < T:
        n = min(mid, T - i)
        w.append((i, n))
        i += n
    return w[:len(w)]
WINS_S = {0: _wins(8, 16), 1: _wins(8, 16), 2: _wins(8, 16)}
for _w in WINS_S.values():
    assert sum(n for _, n in _w) == T
CH = 512                # matmul moving-chunk / PSUM bank width
# all chunks >=256 so the fp32r rank-2 matmul stays at 1 cycle/row
_CW = [512, 512, 512, 512, 512, 384, 256]
CHUNKS = [(sum(_CW[:i]), w) for i, w in enumerate(_CW)]
assert sum(_CW) == SJ
BIG = 1e10

_built = None          # cached compiled Bass program
_last_result = None    # last BassKernelResults (exec_time_ns when traced)
_predicted_ns = None   # Tile cost-model makespan of the per-core program


def _build():
    import concourse.bacc as bacc
    import concourse.mybir as mybir
    import concourse.tile as tile

    f32 = mybir.dt.float32
    f32r = mybir.dt.float32r
    bf16 = mybir.dt.bfloat16
    fp8 = mybir.dt.float8e4
    MIN = mybir.AluOpType.min
    ADD = mybir.AluOpType.add
    DR = mybir.MatmulPerfMode.DoubleRow

    global _predicted_ns
    nc = bacc.Bacc("TRN2", debug=False)

    xt_d = nc.dram_tensor("xt", [QC, 128, NKK, 2, T], fp8, kind="ExternalInput")
    yt_d = nc.dram_tensor("yt", [128, NKK, 2, SJ], fp8, kind="ExternalInput")
    # aux DoubleRow rows (K_p=2, 2 slabs = 4 contraction rows):
    #   (k0,s0): 64 * 64            -> +4096 (exact mean of x2+y2)
    #   (k1,s0): rx[i]/8 * 8        -> +(x2[i]-2048) residual
    #   (k0,s1): 8 * ry[sj]/8       -> +(y2[sj]-2048) residual
    #   (k1,s1): 0
    xa_d = nc.dram_tensor("xa", [QC, 2, 2, T], fp8, kind="ExternalInput")
    ya_d = nc.dram_tensor("ya", [2, 2, SJ], fp8, kind="ExternalInput")
    out_d = nc.dram_tensor("out_cd", [QC, S], f32, kind="ExternalOutput")
    # cost matrices staged pair-major and ZERO-INTERLEAVED: [q, s, i, 2T]
    # holds (0, d) pairs so the DP can run ONE fused scan per row:
    #   step even: state = min(R_prev[c], state) + 0
    #   step odd:  state = min(R_prev[c+1], state) + d_c   (= R_cur[c])
    dsc = nc.dram_tensor("dsc", [QC, S, T, 2 * T], bf16)
    dsc_p = dsc[:].rearrange("q s i j2 -> (q s) i j2")

    with tile.TileContext(nc) as tc:
        with (
            tc.tile_pool(name="const", bufs=1) as constp,
            tc.tile_pool(name="xq", bufs=3) as xqp,
            tc.tile_pool(name="augq", bufs=3) as augqp,
            tc.tile_pool(name="psum", bufs=8, space="PSUM") as psump,
            tc.tile_pool(name="dq", bufs=1) as dqp,
            tc.tile_pool(name="ga", bufs=2) as gap,
            tc.tile_pool(name="gb", bufs=2) as gbp,
            tc.tile_pool(name="gc", bufs=2) as gcp,
            tc.tile_pool(name="dp", bufs=1) as dpp,
        ):
            # q0's operands first on the ACT queue (ahead of the yt pieces).
            xt0_sb = xqp.tile([128, NKK, 2, T], fp8, tag="xt")
            nc.scalar.dma_start(xt0_sb[:], xt_d[QORDER[0]])
            xa0_sb = augqp.tile([2, 2, T], fp8, tag="xa")
            nc.scalar.dma_start(xa0_sb[:], xa_d[QORDER[0]])

            # Resident Y^T (fp8), per K-tile so q0 starts after ~800KB,
            # spread across 3 DMA queues (SP + ACT + Pool/SWDGE) so all of
            # yt lands within ~7us. The Pool sequencer is idle until the
            # wavefront starts, so the SWDGE generation there is free.
            yt_sb = constp.tile([128, NKK, 2, SJ], fp8)
            for k in range(NKK):
                qeng = [nc.sync, nc.scalar, nc.gpsimd][k % 3]
                qeng.dma_start(yt_sb[:, k, :, :], yt_d[:, k, :, :])
            # ya is only 2 partitions deep (12.8KB per partition = 4.9us on
            # one ring): quarter it across rings so q0's aux chunks aren't
            # gated on one long transfer.
            ya_sb = constp.tile([2, 2, SJ], fp8)
            for yi in range(4):
                c0, c1 = yi * (SJ // 4), (yi + 1) * (SJ // 4)
                qeng = [nc.sync, nc.scalar, nc.gpsimd, nc.sync][yi]
                qeng.dma_start(ya_sb[:, :, c0:c1], ya_d[:, :, c0:c1])

            # Two persistent interleaved staging buffers [128, 2*SJ]:
            # odd slots take the PSUM evacuation (strided ACT writes), even
            # slots are zeroed ONCE here (idle DVE) and persist physically.
            dq_bufs = []
            for di in range(2):
                dq_t = dqp.tile([128, 2 * SJ], bf16, tag=f"dq{di}")
                dq_bufs.append(dq_t)
                dq_ev = dq_t[:].rearrange("p (sj two) -> p two sj", two=2)
                for c0, cw in CHUNKS:
                    nc.vector.memset(dq_ev[:, 0, c0:c0 + cw], 0.0)

            # ---- Stage A: cost matrices, one query at a time (k-outer) ----
            xt_tiles = {QORDER[0]: xt0_sb}
            xa_tiles = {QORDER[0]: xa0_sb}

            def prefetch(qn):
                xt_sb_n = xqp.tile([128, NKK, 2, T], fp8, tag="xt")
                nc.sync.dma_start(xt_sb_n[:], xt_d[qn])
                xa_sb_n = augqp.tile([2, 2, T], fp8, tag="xa")
                nc.sync.dma_start(xa_sb_n[:], xa_d[qn])
                xt_tiles[qn] = xt_sb_n
                xa_tiles[qn] = xa_sb_n

            # 2-deep lookahead: q1's operands requested with the yt stream
            prefetch(QORDER[1])
            for qi, q in enumerate(QORDER):
                xt_sb, xa_sb = xt_tiles.pop(q), xa_tiles.pop(q)
                dq_sb = dq_bufs[qi % 2]
                dq_odd = dq_sb[:].rearrange("p (sj two) -> p two sj", two=2)
                # k-OUTER: each fp8 weight tile streams all 7 chunks (7 PSUM
                # banks live), so weight loads amortize 7x; for q0 every
                # arriving yt K-tile immediately feeds all chunks, so q0's
                # matrix completes with the prologue.
                pss = []
                for _ci in range(len(CHUNKS)):
                    ps_t = psump.tile([128, CH], f32, tag="ps")
                    pss.append(ps_t)
                for kk in range(NKK):
                    for ci, (c0, cw) in enumerate(CHUNKS):
                        nc.tensor.matmul(
                            pss[ci][:, :cw],
                            xt_sb[:, kk, :, :],
                            yt_sb[:, kk, :, c0:c0 + cw],
                            start=(kk == 0),
                            stop=False,
                            perf_mode=DR,
                        )
                for ci, (c0, cw) in enumerate(CHUNKS):
                    # fp8 aux rank-4: + 4096 + rx[i] + ry[s,j]
                    nc.tensor.matmul(
                        pss[ci][:, :cw],
                        xa_sb[:, :, :],
                        ya_sb[:, :, c0:c0 + cw],
                        start=False,
                        stop=True,
                        perf_mode=DR,
                    )
                    nc.scalar.copy(dq_odd[:, 1, c0:c0 + cw], pss[ci][:, :cw])
                    if ci == 0 and qi + 2 < QC:
                        # 2-query lookahead on the SP ring, emitted ahead of
                        # this query's dsc writes: the transfer clears the
                        # queue a full query before the PE needs it.
                        prefetch(QORDER[qi + 2])
                    # per-chunk dsc write (chunk widths are whole s-blocks)
                    nc.sync.dma_start(
                        dsc[q, c0 // T:(c0 + cw) // T]
                        .rearrange("s i j2 -> i s j2"),
                        dq_sb[:, 2 * c0:2 * (c0 + cw)]
                        .rearrange("i (s j2) -> i s j2", j2=2 * T))

            # ---- Stage B: hard-DTW wavefront, 3 batched pair-tiles ----
            # One FUSED scan per row (free = 2T): the 3-way min is expanded
            # into two scan steps per cell, data0 = overlapping (R[c], R[c+1])
            # pairs of the interleaved prev rowbuf (hand-built strided AP),
            # data1 = (0, d) pairs from the interleaved gather tiles. No
            # separate mud instruction, one fewer dep link per row.
            out_flat = out_d[:].rearrange("q s -> (q s)")
            T2 = 2 * T

            def fused_row_scan(cur_ap, prev_tile, np_, g_ap):
                # data0: [np, 1+2T) viewed as [np, T, 2] with strides (2, 2)
                a = prev_tile[:np_, 1:T2 + 1].copy()
                ap0 = [list(x) for x in a.ap]
                assert ap0[-1][0] == 1 and ap0[-1][1] == T2, ap0
                a.ap = mybir.VecI64Pair(ap0[:-1] + [[2, T], [2, 2]])
                nc.vector.add_instruction(
                    mybir.InstTensorScalarPtr(
                        name=nc.get_next_instruction_name(),
                        is_tensor_tensor_scan=True,
                        is_scalar_tensor_tensor=True,
                        op0=MIN,
                        op1=ADD,
                        ins=[nc.vector.lower_ap(a),
                             mybir.ImmediateValue(dtype=f32, value=BIG),
                             nc.vector.lower_ap(g_ap)],
                        outs=[nc.vector.lower_ap(cur_ap)],
                    )
                )

            for pt, (p0, np_) in enumerate(PT):
                gpool = [gap, gbp, gcp][pt]

                # interleaved rowbufs: [pad, border, (partial, R) * T, pad]
                r_a = dpp.tile([128, T2 + 4], f32, tag=f"ra{pt}")
                r_b = dpp.tile([128, T2 + 4], f32, tag=f"rb{pt}")
                # row 0 of the R-grid lives in r_a's odd slots: corner 0 at
                # the border slot 1, BIG elsewhere; r_b border = BIG.
                # memsets on Pool: keeps them off the DVE critical chain.
                nc.gpsimd.memset(r_a[:np_, 1:T2 + 2], BIG)
                nc.gpsimd.memset(r_a[:np_, 1:2], 0.0)
                nc.gpsimd.memset(r_b[:np_, 1:2], BIG)

                WINS = WINS_S[pt]
                win_of = {}
                for wi, (w0, wn) in enumerate(WINS):
                    for i in range(w0, w0 + wn):
                        win_of[i] = (wi, w0)
                g_tiles = {}
                for i in range(T):
                    wi, w0 = win_of[i]
                    if i == w0:
                        wn = WINS[wi][1]
                        g_t = gpool.tile([128, 16 * T2], bf16, tag=f"g{pt}")
                        g_tiles[wi] = g_t
                        # One DMA per window (full SDMA-engine spread).
                        # Pool/SWDGE: idle sequencer, not paced by ACT/SP.
                        # Window 0 splits off the final query so the earlier
                        # queries prefetch while the last matrix is still
                        # being written.
                        if pt >= 1 and wi == 0:
                            cut = np_ - S
                            nc.gpsimd.dma_start(
                                g_t[:cut, :wn * T2].rearrange(
                                    "p (w j2) -> p w j2", j2=T2),
                                dsc_p[p0:p0 + cut, w0:w0 + wn, :],
                            )
                            nc.gpsimd.dma_start(
                                g_t[cut:np_, :wn * T2].rearrange(
                                    "p (w j2) -> p w j2", j2=T2),
                                dsc_p[p0 + cut:p0 + np_, w0:w0 + wn, :],
                            )
                        else:
                            nc.gpsimd.dma_start(
                                g_t[:np_, :wn * T2].rearrange(
                                    "p (w j2) -> p w j2", j2=T2),
                                dsc_p[p0:p0 + np_, w0:w0 + wn, :],
                            )
                    g_t = g_tiles[wi]
                    prev, cur = (r_a, r_b) if i % 2 == 0 else (r_b, r_a)
                    fused_row_scan(
                        cur[:np_, 2:T2 + 2], prev, np_,
                        g_t[:np_, (i - w0) * T2:(i - w0 + 1) * T2])
                    if i == 0:
                        # row-0 buffer becomes an interior row: border 0 -> BIG
                        nc.vector.memset(prev[:np_, 1:2], BIG)

                final = r_b if T % 2 == 1 else r_a  # T=128 even -> last cur=r_a
                nc.sync.dma_start(
                    out_flat[p0:p0 + np_], final[:np_, T2 + 1:T2 + 2])

    ents = getattr(tc, "_perfetto_entries", None)
    if ents:
        _predicted_ns = int(max(e[2] for e in ents))
    nc.compile()
    return nc


def _pack_inputs(X, Yf):
    """Host-side packing into the exact SBUF layouts the kernel DMAs 1:1."""
    f8 = ml_dtypes.float8_e4m3
    # xt: [QPAD, 128(dk), NKK, 2, T] = fp8(-2*X)^T, DoubleRow slab layout
    # contraction index d = kk*256 + slab*128 + dk
    Xp = np.zeros((QPAD, T, DD), np.float32)
    Xp[:Q] = X
    xtq = np.ascontiguousarray(
        (-2.0 * Xp).astype(f8).transpose(0, 2, 1)        # [QPAD, DD, T]
        .reshape(QPAD, NKK, 2, 128, T).transpose(0, 3, 1, 2, 4))
    # yt: [128(dk), NKK, 2, SJ] = fp8(Y)^T
    yt = np.ascontiguousarray(
        Yf.astype(f8).transpose(2, 0, 1)                 # [DD, S, T]
        .reshape(NKK, 2, 128, SJ).transpose(2, 0, 1, 3))
    # norms folded into fp8 aux DoubleRow rows:
    # x2+y2 = 4096 (exact, 64*64) + rx + ry residuals (fp8/8, err ~+-8)
    x2 = np.einsum("qtd,qtd->qt", Xp, Xp, dtype=np.float32)  # [QPAD, T]
    y2 = np.einsum("std,std->st", Yf, Yf, dtype=np.float32)  # [S, T]
    xa = np.zeros((QPAD, 2, 2, T), f8)
    xa[:, 0, 0, :] = f8(64.0)
    xa[:, 1, 0, :] = ((x2 - 2048.0) / 8.0).astype(f8)
    xa[:, 0, 1, :] = f8(8.0)
    ya = np.zeros((2, 2, SJ), f8)
    ya[0, 0, :] = f8(64.0)
    ya[1, 0, :] = f8(8.0)
    ya[0, 1, :] = ((y2.reshape(SJ) - 2048.0) / 8.0).astype(f8)
    return xtq, yt, xa, ya


def kernel(support_features, support_labels, target_features, n_classes):
    global _built
    from concourse.bass_utils import run_bass_kernel_spmd

    X = np.asarray(target_features, dtype=np.float32)
    Yf = np.asarray(support_features, dtype=np.float32)
    labels = np.asarray(support_labels)
    ncls = int(np.asarray(n_classes))
    assert X.shape == (Q, T, DD) and Yf.shape == (S, T, DD), (
        f"kernel compiled for fixed shapes; got {X.shape}, {Yf.shape}")

    xtq, yt, xa, ya = _pack_inputs(X, Yf)

    if _built is None:
        _built = _build()
    nc = _built

    in_maps = [
        {
            "xt": np.ascontiguousarray(xtq[c * QC:(c + 1) * QC]),
            "yt": yt,
            "xa": np.ascontiguousarray(xa[c * QC:(c + 1) * QC]),
            "ya": ya,
        }
        for c in range(NCORES)
    ]
    res = run_bass_kernel_spmd(nc, in_maps, list(range(NCORES)))
    global _last_result
    _last_result = res
    cum = np.concatenate([res.results[c]["out_cd"] for c in range(NCORES)])[:Q]

    onehot = (labels[:, None] == np.arange(ncls)[None, :]).astype(np.float32)
    counts = np.maximum(onehot.sum(axis=0), 1.0).astype(np.float32)
    logits = -(cum.astype(np.float32) @ onehot) / counts
    return logits.astype(np.float32)


# revision 8
# speedup vs baseline: 2.2162x; 1.0004x over previous
"""Trainium2 Bass kernel: CNN-feature SoftDTW few-shot classifier.

Computes, for Q=100 query sequences and S=25 support sequences (T=128 steps,
D=2048 features): pairwise squared-euclidean cost matrices, soft-DTW alignment
cost per (query, support) pair, then per-class mean distances -> logits.

Key numerical fact: with gamma=0.1 and cost magnitudes ~4096, the reference's
fp32 softmin is bitwise the hard min (exp((m-x)/gamma) underflows for every
non-minimal branch), so the DP is computed with min/add only.

Design (385us baseline -> 174us):
  - fp8e4m3 DoubleRow matmul (0.5 cy/row, K=256/instr) for xy = (-2X)@Y^T:
    4x the bf16 rate. k-outer order, 7 PSUM banks live, so each weight tile
    streams all 3200 columns.
  - x2[i] + y2[s,j] folded into ONE extra fp8 DoubleRow instruction per
    chunk via aux contraction rows: 64*64 (=4096, the exact mean) plus
    (x2-2048)/8 * 8 and 8 * (y2-2048)/8 residuals (+-8 quantization on a
    ~4096 cell; path-sum error ~1e-4 of output scale, threshold 2e-2).
  - DP: ONE fused tensor_tensor_scan per DTW row (free = 2T): the 3-way min
    is expanded into two scan steps per cell,
        state = min(R_prev[c],   state) + 0
        state = min(R_prev[c+1], state) + d_c      (= R_cur[c])
    data0 is a hand-built overlapping strided AP ([j:2,T][slot:2,2]) over
    the interleaved fp32 rowbuf; data1 interleaves zeros with bf16 costs.
    The zero-interleaving rides for free: PSUM evacuation writes odd slots
    of a persistently even-zeroed staging buffer, and the (0,d) pairs flow
    through DRAM and the gather DMA contiguously. This removes the separate
    min(up,diag) instruction AND one dependency link per row: 327ns/row
    engine, 427ns/row serial-chain latency.
  - 3 pair streams (q0-2 | q3-7 | q8-12, pairs on partitions); stage A
    processes slots in order [0-2, 8-12, 3-7] so the two big streams'
    data gates land earliest. Gathers via Pool/SWDGE windows (first window
    8 rows) with per-stream double-buffered pools.

Sharding: data-parallel over queries, 13 per core (Q padded 100->104),
supports replicated. Host: fp8 packing/transposes, x2/y2 sums, final
class-mean logits.
"""

import sys

for _p in ("/opt/trn_rl_repo",):
    if _p not in sys.path:
        sys.path.insert(0, _p)

import numpy as np
import ml_dtypes

# Problem shape (hardcoded: harness runs kernel.py standalone)
Q, S, T, DD = 100, 25, 128, 2048
NCORES = 8
QC = 13                 # queries per core; Q padded to 104
QPAD = QC * NCORES
NKK = DD // 256         # 8 fp8 DoubleRow contraction tiles (256 deep each)
SJ = S * T              # 3200 = flattened (support, j)
B = QC * S              # 325 pairs per core
# DP pair-tile streams aligned to query boundaries (offset, count):
# a stream's first row can run as soon as its LAST query's cost matrix is
# in DRAM. 3/5/5 queries: the DVE wavefront starts at q2 instead of q4.
PT = [(0, 75), (75, 125), (200, 125)]   # q0-2 | q3-7 | q8-12
# stage-A processing order of the 13 query slots. Streams gate on their
# LAST slot: s1 on slot 2, s2 on slot 7, s3 on slot 12.
# Process s3's queries (slots 8-12) right after s1's gate: the wavefront
# stream gates land at positions (3, 13, 8) of 13 - the two big streams
# start as early as possible and the middle one last.
QORDER = [0, 1, 2, 8, 9, 10, 11, 12, 3, 4, 5, 6, 7]
# DP row-windows per gather DMA: small first window so each stream's
# wavefront starts ~2us earlier (less gather gen+transfer to wait on).
# Stream 2 (middle) uses smaller windows + a single-buffered gather pool:
# its rows are RELEASED gradually, so the greedy scheduler cannot drain
# stream 2 early and leave stream 3 alone (at 66% serial rate) in the tail.
def _wins(first, mid):
    w, i = [(0, first)], first
    while i < T:
        n = min(mid, T - i)
        w.append((i, n))
        i += n
    return w[:len(w)]
WINS_S = {0: _wins(8, 16), 1: _wins(8, 16), 2: _wins(8, 16)}
for _w in WINS_S.values():
    assert sum(n for _, n in _w) == T
CH = 512                # matmul moving-chunk / PSUM bank width
# all chunks >=256 so the fp32r rank-2 matmul stays at 1 cycle/row
_CW = [512, 512, 512, 512, 512, 384, 256]
CHUNKS = [(sum(_CW[:i]), w) for i, w in enumerate(_CW)]
assert sum(_CW) == SJ
BIG = 1e10

_built = None          # cached compiled Bass program
_last_result = None    # last BassKernelResults (exec_time_ns when traced)
_predicted_ns = None   # Tile cost-model makespan of the per-core program


def _build():
    import concourse.bacc as bacc
    import concourse.mybir as mybir
    import concourse.tile as tile

    f32 = mybir.dt.float32
    f32r = mybir.dt.float32r
    bf16 = mybir.dt.bfloat16
    fp8 = mybir.dt.float8e4
    MIN = mybir.AluOpType.min
    ADD = mybir.AluOpType.add
    DR = mybir.MatmulPerfMode.DoubleRow

    global _predicted_ns
    nc = bacc.Bacc("TRN2", debug=False)

    xt_d = nc.dram_tensor("xt", [QC, 128, NKK, 2, T], fp8, kind="ExternalInput")
    yt_d = nc.dram_tensor("yt", [128, NKK, 2, SJ], fp8, kind="ExternalInput")
    # aux DoubleRow rows (K_p=2, 2 slabs = 4 contraction rows):
    #   (k0,s0): 64 * 64            -> +4096 (exact mean of x2+y2)
    #   (k1,s0): rx[i]/8 * 8        -> +(x2[i]-2048) residual
    #   (k0,s1): 8 * ry[sj]/8       -> +(y2[sj]-2048) residual
    #   (k1,s1): 0
    xa_d = nc.dram_tensor("xa", [QC, 2, 2, T], fp8, kind="ExternalInput")
    ya_d = nc.dram_tensor("ya", [2, 2, SJ], fp8, kind="ExternalInput")
    out_d = nc.dram_tensor("out_cd", [QC, S], f32, kind="ExternalOutput")
    # cost matrices staged pair-major and ZERO-INTERLEAVED: [q, s, i, 2T]
    # holds (0, d) pairs so the DP can run ONE fused scan per row:
    #   step even: state = min(R_prev[c], state) + 0
    #   step odd:  state = min(R_prev[c+1], state) + d_c   (= R_cur[c])
    dsc = nc.dram_tensor("dsc", [QC, S, T, 2 * T], bf16)
    dsc_p = dsc[:].rearrange("q s i j2 -> (q s) i j2")

    with tile.TileContext(nc) as tc:
        with (
            tc.tile_pool(name="const", bufs=1) as constp,
            tc.tile_pool(name="xq", bufs=3) as xqp,
            tc.tile_pool(name="augq", bufs=3) as augqp,
            tc.tile_pool(name="psum", bufs=8, space="PSUM") as psump,
            tc.tile_pool(name="dq", bufs=1) as dqp,
            tc.tile_pool(name="ga", bufs=2) as gap,
            tc.tile_pool(name="gb", bufs=2) as gbp,
            tc.tile_pool(name="gc", bufs=2) as gcp,
            tc.tile_pool(name="dp", bufs=1) as dpp,
        ):
            # q0's operands first on the ACT queue (ahead of the yt pieces).
            xt0_sb = xqp.tile([128, NKK, 2, T], fp8, tag="xt")
            nc.scalar.dma_start(xt0_sb[:], xt_d[QORDER[0]])
            xa0_sb = augqp.tile([2, 2, T], fp8, tag="xa")
            nc.scalar.dma_start(xa0_sb[:], xa_d[QORDER[0]])

            # Resident Y^T (fp8), per K-tile so q0 starts after ~800KB,
            # spread across 3 DMA queues (SP + ACT + Pool/SWDGE) so all of
            # yt lands within ~7us. The Pool sequencer is idle until the
            # wavefront starts, so the SWDGE generation there is free.
            yt_sb = constp.tile([128, NKK, 2, SJ], fp8)
            for k in range(NKK):
                qeng = [nc.sync, nc.scalar, nc.gpsimd][k % 3]
                qeng.dma_start(yt_sb[:, k, :, :], yt_d[:, k, :, :])
            # ya is only 2 partitions deep (12.8KB per partition = 4.9us on
            # one ring): quarter it across rings so q0's aux chunks aren't
            # gated on one long transfer.
            ya_sb = constp.tile([2, 2, SJ], fp8)
            for yi in range(4):
                c0, c1 = yi * (SJ // 4), (yi + 1) * (SJ // 4)
                qeng = [nc.sync, nc.scalar, nc.gpsimd, nc.sync][yi]
                qeng.dma_start(ya_sb[:, :, c0:c1], ya_d[:, :, c0:c1])

            # Two persistent interleaved staging buffers [128, 2*SJ]:
            # odd slots take the PSUM evacuation (strided ACT writes), even
            # slots are zeroed ONCE here (idle DVE) and persist physically.
            dq_bufs = []
            for di in range(3):
                dq_t = dqp.tile([128, 2 * SJ], bf16, tag=f"dq{di}")
                dq_bufs.append(dq_t)
                dq_ev = dq_t[:].rearrange("p (sj two) -> p two sj", two=2)
                for c0, cw in CHUNKS:
                    nc.vector.memset(dq_ev[:, 0, c0:c0 + cw], 0.0)

            # PE p-state warm-up: the tensor engine needs ~3us of continuous
            # execution to reach 2.4GHz (0.65/1.2GHz below that). Chew junk
            # matmuls on the already-resident xa0 tile into a scratch bank
            # from t~1.3us so q0's real matmuls run at full clock.
            ps_warm = psump.tile([128, CH], f32, tag="ps")
            for wi in range(40):
                nc.tensor.matmul(
                    ps_warm[:, :128],
                    xa0_sb[:, :, :],
                    xa0_sb[:, :, :],
                    start=(wi == 0),
                    stop=(wi == 39),
                    perf_mode=DR,
                )

            # ---- Stage A: cost matrices, one query at a time (k-outer) ----
            xt_tiles = {QORDER[0]: xt0_sb}
            xa_tiles = {QORDER[0]: xa0_sb}

            def prefetch(qn):
                xt_sb_n = xqp.tile([128, NKK, 2, T], fp8, tag="xt")
                nc.sync.dma_start(xt_sb_n[:], xt_d[qn])
                xa_sb_n = augqp.tile([2, 2, T], fp8, tag="xa")
                nc.sync.dma_start(xa_sb_n[:], xa_d[qn])
                xt_tiles[qn] = xt_sb_n
                xa_tiles[qn] = xa_sb_n

            # 2-deep lookahead: q1's operands requested with the yt stream
            prefetch(QORDER[1])
            for qi, q in enumerate(QORDER):
                xt_sb, xa_sb = xt_tiles.pop(q), xa_tiles.pop(q)
                dq_sb = dq_bufs[qi % 3]
                dq_odd = dq_sb[:].rearrange("p (sj two) -> p two sj", two=2)
                # k-OUTER: each fp8 weight tile streams all 7 chunks (7 PSUM
                # banks live), so weight loads amortize 7x; for q0 every
                # arriving yt K-tile immediately feeds all chunks, so q0's
                # matrix completes with the prologue.
                pss = []
                for _ci in range(len(CHUNKS)):
                    ps_t = psump.tile([128, CH], f32, tag="ps")
                    pss.append(ps_t)
                for kk in range(NKK):
                    for ci, (c0, cw) in enumerate(CHUNKS):
                        nc.tensor.matmul(
                            pss[ci][:, :cw],
                            xt_sb[:, kk, :, :],
                            yt_sb[:, kk, :, c0:c0 + cw],
                            start=(kk == 0),
                            stop=False,
                            perf_mode=DR,
                        )
                for ci, (c0, cw) in enumerate(CHUNKS):
                    # fp8 aux rank-4: + 4096 + rx[i] + ry[s,j]
                    nc.tensor.matmul(
                        pss[ci][:, :cw],
                        xa_sb[:, :, :],
                        ya_sb[:, :, c0:c0 + cw],
                        start=False,
                        stop=True,
                        perf_mode=DR,
                    )
                    nc.scalar.copy(dq_odd[:, 1, c0:c0 + cw], pss[ci][:, :cw])
                    if ci == 0 and qi + 2 < QC:
                        # 2-query lookahead on the SP ring, emitted ahead of
                        # this query's dsc writes: the transfer clears the
                        # queue a full query before the PE needs it.
                        prefetch(QORDER[qi + 2])
                    # per-chunk dsc write (chunk widths are whole s-blocks)
                    nc.sync.dma_start(
                        dsc[q, c0 // T:(c0 + cw) // T]
                        .rearrange("s i j2 -> i s j2"),
                        dq_sb[:, 2 * c0:2 * (c0 + cw)]
                        .rearrange("i (s j2) -> i s j2", j2=2 * T))

            # ---- Stage B: hard-DTW wavefront, 3 batched pair-tiles ----
            # One FUSED scan per row (free = 2T): the 3-way min is expanded
            # into two scan steps per cell, data0 = overlapping (R[c], R[c+1])
            # pairs of the interleaved prev rowbuf (hand-built strided AP),
            # data1 = (0, d) pairs from the interleaved gather tiles. No
            # separate mud instruction, one fewer dep link per row.
            out_flat = out_d[:].rearrange("q s -> (q s)")
            T2 = 2 * T

            def fused_row_scan(cur_ap, prev_tile, np_, g_ap):
                # data0: [np, 1+2T) viewed as [np, T, 2] with strides (2, 2)
                a = prev_tile[:np_, 1:T2 + 1].copy()
                ap0 = [list(x) for x in a.ap]
                assert ap0[-1][0] == 1 and ap0[-1][1] == T2, ap0
                a.ap = mybir.VecI64Pair(ap0[:-1] + [[2, T], [2, 2]])
                nc.vector.add_instruction(
                    mybir.InstTensorScalarPtr(
                        name=nc.get_next_instruction_name(),
                        is_tensor_tensor_scan=True,
                        is_scalar_tensor_tensor=True,
                        op0=MIN,
                        op1=ADD,
                        ins=[nc.vector.lower_ap(a),
                             mybir.ImmediateValue(dtype=f32, value=BIG),
                             nc.vector.lower_ap(g_ap)],
                        outs=[nc.vector.lower_ap(cur_ap)],
                    )
                )

            for pt, (p0, np_) in enumerate(PT):
                gpool = [gap, gbp, gcp][pt]

                # interleaved rowbufs: [pad, border, (partial, R) * T, pad]
                r_a = dpp.tile([128, T2 + 4], f32, tag=f"ra{pt}")
                r_b = dpp.tile([128, T2 + 4], f32, tag=f"rb{pt}")
                # row 0 of the R-grid lives in r_a's odd slots: corner 0 at
                # the border slot 1, BIG elsewhere; r_b border = BIG.
                # memsets on Pool: keeps them off the DVE critical chain.
                nc.gpsimd.memset(r_a[:np_, 1:T2 + 2], BIG)
                nc.gpsimd.memset(r_a[:np_, 1:2], 0.0)
                nc.gpsimd.memset(r_b[:np_, 1:2], BIG)

                WINS = WINS_S[pt]
                win_of = {}
                for wi, (w0, wn) in enumerate(WINS):
                    for i in range(w0, w0 + wn):
                        win_of[i] = (wi, w0)
                g_tiles = {}
                for i in range(T):
                    wi, w0 = win_of[i]
                    if i == w0:
                        wn = WINS[wi][1]
                        g_t = gpool.tile([128, 16 * T2], bf16, tag=f"g{pt}")
                        g_tiles[wi] = g_t
                        # One DMA per window (full SDMA-engine spread).
                        # Pool/SWDGE: idle sequencer, not paced by ACT/SP.
                        # Window 0 splits off the final query so the earlier
                        # queries prefetch while the last matrix is still
                        # being written.
                        if pt >= 1 and wi == 0:
                            cut = np_ - S
                            nc.gpsimd.dma_start(
                                g_t[:cut, :wn * T2].rearrange(
                                    "p (w j2) -> p w j2", j2=T2),
                                dsc_p[p0:p0 + cut, w0:w0 + wn, :],
                            )
                            nc.gpsimd.dma_start(
                                g_t[cut:np_, :wn * T2].rearrange(
                                    "p (w j2) -> p w j2", j2=T2),
                                dsc_p[p0 + cut:p0 + np_, w0:w0 + wn, :],
                            )
                        else:
                            nc.gpsimd.dma_start(
                                g_t[:np_, :wn * T2].rearrange(
                                    "p (w j2) -> p w j2", j2=T2),
                                dsc_p[p0:p0 + np_, w0:w0 + wn, :],
                            )
                    g_t = g_tiles[wi]
                    prev, cur = (r_a, r_b) if i % 2 == 0 else (r_b, r_a)
                    fused_row_scan(
                        cur[:np_, 2:T2 + 2], prev, np_,
                        g_t[:np_, (i - w0) * T2:(i - w0 + 1) * T2])
                    if i == 0:
                        # row-0 buffer becomes an interior row: border 0 -> BIG
                        nc.vector.memset(prev[:np_, 1:2], BIG)

                final = r_b if T % 2 == 1 else r_a  # T=128 even -> last cur=r_a
                nc.sync.dma_start(
                    out_flat[p0:p0 + np_], final[:np_, T2 + 1:T2 + 2])

    ents = getattr(tc, "_perfetto_entries", None)
    if ents:
        _predicted_ns = int(max(e[2] for e in ents))
    nc.compile()
    return nc


def _pack_inputs(X, Yf):
    """Host-side packing into the exact SBUF layouts the kernel DMAs 1:1."""
    f8 = ml_dtypes.float8_e4m3
    # xt: [QPAD, 128(dk), NKK, 2, T] = fp8(-2*X)^T, DoubleRow slab layout
    # contraction index d = kk*256 + slab*128 + dk
    Xp = np.zeros((QPAD, T, DD), np.float32)
    Xp[:Q] = X
    xtq = np.ascontiguousarray(
        (-2.0 * Xp).astype(f8).transpose(0, 2, 1)        # [QPAD, DD, T]
        .reshape(QPAD, NKK, 2, 128, T).transpose(0, 3, 1, 2, 4))
    # yt: [128(dk), NKK, 2, SJ] = fp8(Y)^T
    yt = np.ascontiguousarray(
        Yf.astype(f8).transpose(2, 0, 1)                 # [DD, S, T]
        .reshape(NKK, 2, 128, SJ).transpose(2, 0, 1, 3))
    # norms folded into fp8 aux DoubleRow rows:
    # x2+y2 = 4096 (exact, 64*64) + rx + ry residuals (fp8/8, err ~+-8)
    x2 = np.einsum("qtd,qtd->qt", Xp, Xp, dtype=np.float32)  # [QPAD, T]
    y2 = np.einsum("std,std->st", Yf, Yf, dtype=np.float32)  # [S, T]
    xa = np.zeros((QPAD, 2, 2, T), f8)
    xa[:, 0, 0, :] = f8(64.0)
    xa[:, 1, 0, :] = ((x2 - 2048.0) / 8.0).astype(f8)
    xa[:, 0, 1, :] = f8(8.0)
    ya = np.zeros((2, 2, SJ), f8)
    ya[0, 0, :] = f8(64.0)
    ya[1, 0, :] = f8(8.0)
    ya[0, 1, :] = ((y2.reshape(SJ) - 2048.0) / 8.0).astype(f8)
    return xtq, yt, xa, ya


def kernel(support_features, support_labels, target_features, n_classes):
    global _built
    from concourse.bass_utils import run_bass_kernel_spmd

    X = np.asarray(target_features, dtype=np.float32)
    Yf = np.asarray(support_features, dtype=np.float32)
    labels = np.asarray(support_labels)
    ncls = int(np.asarray(n_classes))
    assert X.shape == (Q, T, DD) and Yf.shape == (S, T, DD), (
        f"kernel compiled for fixed shapes; got {X.shape}, {Yf.shape}")

    xtq, yt, xa, ya = _pack_inputs(X, Yf)

    if _built is None:
        _built = _build()
    nc = _built

    in_maps = [
        {
            "xt": np.ascontiguousarray(xtq[c * QC:(c + 1) * QC]),
            "yt": yt,
            "xa": np.ascontiguousarray(xa[c * QC:(c + 1) * QC]),
            "ya": ya,
        }
        for c in range(NCORES)
    ]
    res = run_bass_kernel_spmd(nc, in_maps, list(range(NCORES)))
    global _last_result
    _last_result = res
    cum = np.concatenate([res.results[c]["out_cd"] for c in range(NCORES)])[:Q]

    onehot = (labels[:, None] == np.arange(ncls)[None, :]).astype(np.float32)
    counts = np.maximum(onehot.sum(axis=0), 1.0).astype(np.float32)
    logits = -(cum.astype(np.float32) @ onehot) / counts
    return logits.astype(np.float32)


# revision 9
# speedup vs baseline: 2.2199x; 1.0017x over previous
"""Trainium2 Bass kernel: CNN-feature SoftDTW few-shot classifier.

Computes, for Q=100 query sequences and S=25 support sequences (T=128 steps,
D=2048 features): pairwise squared-euclidean cost matrices, soft-DTW alignment
cost per (query, support) pair, then per-class mean distances -> logits.

Key numerical fact: with gamma=0.1 and cost magnitudes ~4096, the reference's
fp32 softmin is bitwise the hard min (exp((m-x)/gamma) underflows for every
non-minimal branch), so the DP is computed with min/add only.

Design (385us baseline -> 174us):
  - fp8e4m3 DoubleRow matmul (0.5 cy/row, K=256/instr) for xy = (-2X)@Y^T:
    4x the bf16 rate. k-outer order, 7 PSUM banks live, so each weight tile
    streams all 3200 columns.
  - x2[i] + y2[s,j] folded into ONE extra fp8 DoubleRow instruction per
    chunk via aux contraction rows: 64*64 (=4096, the exact mean) plus
    (x2-2048)/8 * 8 and 8 * (y2-2048)/8 residuals (+-8 quantization on a
    ~4096 cell; path-sum error ~1e-4 of output scale, threshold 2e-2).
  - DP: ONE fused tensor_tensor_scan per DTW row (free = 2T): the 3-way min
    is expanded into two scan steps per cell,
        state = min(R_prev[c],   state) + 0
        state = min(R_prev[c+1], state) + d_c      (= R_cur[c])
    data0 is a hand-built overlapping strided AP ([j:2,T][slot:2,2]) over
    the interleaved fp32 rowbuf; data1 interleaves zeros with bf16 costs.
    The zero-interleaving rides for free: PSUM evacuation writes odd slots
    of a persistently even-zeroed staging buffer, and the (0,d) pairs flow
    through DRAM and the gather DMA contiguously. This removes the separate
    min(up,diag) instruction AND one dependency link per row: 327ns/row
    engine, 427ns/row serial-chain latency.
  - 3 pair streams (q0-2 | q3-7 | q8-12, pairs on partitions); stage A
    processes slots in order [0-2, 8-12, 3-7] so the two big streams'
    data gates land earliest. Gathers via Pool/SWDGE windows (first window
    8 rows) with per-stream double-buffered pools.

Sharding: data-parallel over queries, 13 per core (Q padded 100->104),
supports replicated. Host: fp8 packing/transposes, x2/y2 sums, final
class-mean logits.
"""

import sys

for _p in ("/opt/trn_rl_repo",):
    if _p not in sys.path:
        sys.path.insert(0, _p)

import numpy as np
import ml_dtypes

# Problem shape (hardcoded: harness runs kernel.py standalone)
Q, S, T, DD = 100, 25, 128, 2048
NCORES = 8
QC = 13                 # queries per core; Q padded to 104
QPAD = QC * NCORES
NKK = DD // 256         # 8 fp8 DoubleRow contraction tiles (256 deep each)
SJ = S * T              # 3200 = flattened (support, j)
B = QC * S              # 325 pairs per core
# DP pair-tile streams aligned to query boundaries (offset, count):
# a stream's first row can run as soon as its LAST query's cost matrix is
# in DRAM. 3/5/5 queries: the DVE wavefront starts at q2 instead of q4.
PT = [(0, 75), (75, 125), (200, 125)]   # q0-2 | q3-7 | q8-12
# stage-A processing order of the 13 query slots. Streams gate on their
# LAST slot: s1 on slot 2, s2 on slot 7, s3 on slot 12.
# Process s3's queries (slots 8-12) right after s1's gate: the wavefront
# stream gates land at positions (3, 13, 8) of 13 - the two big streams
# start as early as possible and the middle one last.
QORDER = [0, 1, 2, 8, 9, 10, 11, 12, 3, 4, 5, 6, 7]
# DP row-windows per gather DMA: small first window so each stream's
# wavefront starts ~2us earlier (less gather gen+transfer to wait on).
# Stream 2 (middle) uses smaller windows + a single-buffered gather pool:
# its rows are RELEASED gradually, so the greedy scheduler cannot drain
# stream 2 early and leave stream 3 alone (at 66% serial rate) in the tail.
def _wins(first, mid):
    w, i = [(0, first)], first
    while i < T:
        n = min(mid, T - i)
        w.append((i, n))
        i += n
    return w[:len(w)]
WINS_S = {0: _wins(8, 16), 1: _wins(8, 16), 2: _wins(8, 16)}
for _w in WINS_S.values():
    assert sum(n for _, n in _w) == T
CH = 512                # matmul moving-chunk / PSUM bank width
# all chunks >=256 so the fp32r rank-2 matmul stays at 1 cycle/row
_CW = [512, 512, 512, 512, 512, 384, 256]
CHUNKS = [(sum(_CW[:i]), w) for i, w in enumerate(_CW)]
assert sum(_CW) == SJ
BIG = 1e10

_built = None          # cached compiled Bass program
_last_result = None    # last BassKernelResults (exec_time_ns when traced)
_predicted_ns = None   # Tile cost-model makespan of the per-core program


def _build():
    import concourse.bacc as bacc
    import concourse.mybir as mybir
    import concourse.tile as tile

    f32 = mybir.dt.float32
    f32r = mybir.dt.float32r
    bf16 = mybir.dt.bfloat16
    fp8 = mybir.dt.float8e4
    MIN = mybir.AluOpType.min
    ADD = mybir.AluOpType.add
    DR = mybir.MatmulPerfMode.DoubleRow

    global _predicted_ns
    nc = bacc.Bacc("TRN2", debug=False)

    xt_d = nc.dram_tensor("xt", [QC, 128, NKK, 2, T], fp8, kind="ExternalInput")
    yt_d = nc.dram_tensor("yt", [128, NKK, 2, SJ], fp8, kind="ExternalInput")
    # aux DoubleRow rows (K_p=2, 2 slabs = 4 contraction rows):
    #   (k0,s0): 64 * 64            -> +4096 (exact mean of x2+y2)
    #   (k1,s0): rx[i]/8 * 8        -> +(x2[i]-2048) residual
    #   (k0,s1): 8 * ry[sj]/8       -> +(y2[sj]-2048) residual
    #   (k1,s1): 0
    xa_d = nc.dram_tensor("xa", [QC, 2, 2, T], fp8, kind="ExternalInput")
    ya_d = nc.dram_tensor("ya", [2, 2, SJ], fp8, kind="ExternalInput")
    out_d = nc.dram_tensor("out_cd", [QC, S], f32, kind="ExternalOutput")
    # cost matrices staged pair-major and ZERO-INTERLEAVED: [q, s, i, 2T]
    # holds (0, d) pairs so the DP can run ONE fused scan per row:
    #   step even: state = min(R_prev[c], state) + 0
    #   step odd:  state = min(R_prev[c+1], state) + d_c   (= R_cur[c])
    dsc = nc.dram_tensor("dsc", [QC, S, T, 2 * T], bf16)
    dsc_p = dsc[:].rearrange("q s i j2 -> (q s) i j2")

    with tile.TileContext(nc) as tc:
        with (
            tc.tile_pool(name="const", bufs=1) as constp,
            tc.tile_pool(name="xq", bufs=3) as xqp,
            tc.tile_pool(name="augq", bufs=3) as augqp,
            tc.tile_pool(name="psum", bufs=8, space="PSUM") as psump,
            tc.tile_pool(name="dq", bufs=1) as dqp,
            tc.tile_pool(name="ga", bufs=2) as gap,
            tc.tile_pool(name="gb", bufs=2) as gbp,
            tc.tile_pool(name="gc", bufs=2) as gcp,
            tc.tile_pool(name="dp", bufs=1) as dpp,
        ):
            # q0's operands first on the ACT queue (ahead of the yt pieces).
            xt0_sb = xqp.tile([128, NKK, 2, T], fp8, tag="xt")
            nc.scalar.dma_start(xt0_sb[:], xt_d[QORDER[0]])
            xa0_sb = augqp.tile([2, 2, T], fp8, tag="xa")
            nc.scalar.dma_start(xa0_sb[:], xa_d[QORDER[0]])

            # Resident Y^T (fp8), per K-tile so q0 starts after ~800KB,
            # spread across 3 DMA queues (SP + ACT + Pool/SWDGE) so all of
            # yt lands within ~7us. The Pool sequencer is idle until the
            # wavefront starts, so the SWDGE generation there is free.
            yt_sb = constp.tile([128, NKK, 2, SJ], fp8)
            for k in range(NKK):
                qeng = [nc.sync, nc.scalar, nc.gpsimd][k % 3]
                qeng.dma_start(yt_sb[:, k, :, :], yt_d[:, k, :, :])
            # ya is only 2 partitions deep (12.8KB per partition = 4.9us on
            # one ring): quarter it across rings so q0's aux chunks aren't
            # gated on one long transfer.
            ya_sb = constp.tile([2, 2, SJ], fp8)
            for yi in range(4):
                c0, c1 = yi * (SJ // 4), (yi + 1) * (SJ // 4)
                qeng = [nc.sync, nc.scalar, nc.gpsimd, nc.sync][yi]
                qeng.dma_start(ya_sb[:, :, c0:c1], ya_d[:, :, c0:c1])

            # Two persistent interleaved staging buffers [128, 2*SJ]:
            # odd slots take the PSUM evacuation (strided ACT writes), even
            # slots are zeroed ONCE here (idle DVE) and persist physically.
            dq_bufs = []
            for di in range(3):
                dq_t = dqp.tile([128, 2 * SJ], bf16, tag=f"dq{di}")
                dq_bufs.append(dq_t)
                dq_ev = dq_t[:].rearrange("p (sj two) -> p two sj", two=2)
                for c0, cw in CHUNKS:
                    nc.vector.memset(dq_ev[:, 0, c0:c0 + cw], 0.0)

            # PE p-state warm-up: the tensor engine needs ~3us of continuous
            # execution to reach 2.4GHz (0.65/1.2GHz below that). Chew junk
            # matmuls on the already-resident xa0 tile into a scratch bank
            # from t~1.3us so q0's real matmuls run at full clock.
            ps_warm = psump.tile([128, CH], f32, tag="ps")
            for wi in range(40):
                nc.tensor.matmul(
                    ps_warm[:, :128],
                    xa0_sb[:, :, :],
                    xa0_sb[:, :, :],
                    start=(wi == 0),
                    stop=(wi == 39),
                    perf_mode=DR,
                )

            # ---- Stage A: cost matrices, one query at a time (k-outer) ----
            xt_tiles = {QORDER[0]: xt0_sb}
            xa_tiles = {QORDER[0]: xa0_sb}

            def prefetch(qn):
                xt_sb_n = xqp.tile([128, NKK, 2, T], fp8, tag="xt")
                nc.sync.dma_start(xt_sb_n[:], xt_d[qn])
                xa_sb_n = augqp.tile([2, 2, T], fp8, tag="xa")
                nc.sync.dma_start(xa_sb_n[:], xa_d[qn])
                xt_tiles[qn] = xt_sb_n
                xa_tiles[qn] = xa_sb_n

            # 2-deep lookahead: q1's operands requested with the yt stream
            prefetch(QORDER[1])
            for qi, q in enumerate(QORDER):
                xt_sb, xa_sb = xt_tiles.pop(q), xa_tiles.pop(q)
                dq_sb = dq_bufs[qi % 3]
                dq_odd = dq_sb[:].rearrange("p (sj two) -> p two sj", two=2)
                # k-OUTER: each fp8 weight tile streams all 7 chunks (7 PSUM
                # banks live), so weight loads amortize 7x; for q0 every
                # arriving yt K-tile immediately feeds all chunks, so q0's
                # matrix completes with the prologue.
                pss = []
                for _ci in range(len(CHUNKS)):
                    ps_t = psump.tile([128, CH], f32, tag="ps")
                    pss.append(ps_t)
                # Stream-gate queries run chunk-major (k-inner): each chunk
                # finishes early so its evacuation overlaps the remaining
                # matmuls - the last evac lands ~2.4us sooner, which is on
                # the critical path of the stream's first gather. Other
                # queries stay k-outer (q0's matrix completes while the yt
                # K-tiles stream in).
                gate = True
                if gate:
                    for ci, (c0, cw) in enumerate(CHUNKS):
                        for kk in range(NKK):
                            nc.tensor.matmul(
                                pss[ci][:, :cw],
                                xt_sb[:, kk, :, :],
                                yt_sb[:, kk, :, c0:c0 + cw],
                                start=(kk == 0),
                                stop=False,
                                perf_mode=DR,
                            )
                        nc.tensor.matmul(
                            pss[ci][:, :cw],
                            xa_sb[:, :, :],
                            ya_sb[:, :, c0:c0 + cw],
                            start=False,
                            stop=True,
                            perf_mode=DR,
                        )
                        nc.scalar.copy(
                            dq_odd[:, 1, c0:c0 + cw], pss[ci][:, :cw])
                        nc.sync.dma_start(
                            dsc[q, c0 // T:(c0 + cw) // T]
                            .rearrange("s i j2 -> i s j2"),
                            dq_sb[:, 2 * c0:2 * (c0 + cw)]
                            .rearrange("i (s j2) -> i s j2", j2=2 * T))
                        if ci == 0 and qi + 2 < QC:
                            prefetch(QORDER[qi + 2])
                    continue
                if True:
                    for kk in range(NKK):
                        for ci, (c0, cw) in enumerate(CHUNKS):
                            nc.tensor.matmul(
                                pss[ci][:, :cw],
                                xt_sb[:, kk, :, :],
                                yt_sb[:, kk, :, c0:c0 + cw],
                                start=(kk == 0),
                                stop=False,
                                perf_mode=DR,
                            )
                for ci, (c0, cw) in enumerate(CHUNKS):
                    # fp8 aux rank-4: + 4096 + rx[i] + ry[s,j]
                    nc.tensor.matmul(
                        pss[ci][:, :cw],
                        xa_sb[:, :, :],
                        ya_sb[:, :, c0:c0 + cw],
                        start=False,
                        stop=True,
                        perf_mode=DR,
                    )
                    nc.scalar.copy(dq_odd[:, 1, c0:c0 + cw], pss[ci][:, :cw])
                    if ci == 0 and qi + 2 < QC:
                        # 2-query lookahead on the SP ring, emitted ahead of
                        # this query's dsc writes: the transfer clears the
                        # queue a full query before the PE needs it.
                        prefetch(QORDER[qi + 2])
                    # per-chunk dsc write (chunk widths are whole s-blocks)
                    nc.sync.dma_start(
                        dsc[q, c0 // T:(c0 + cw) // T]
                        .rearrange("s i j2 -> i s j2"),
                        dq_sb[:, 2 * c0:2 * (c0 + cw)]
                        .rearrange("i (s j2) -> i s j2", j2=2 * T))

            # ---- Stage B: hard-DTW wavefront, 3 batched pair-tiles ----
            # One FUSED scan per row (free = 2T): the 3-way min is expanded
            # into two scan steps per cell, data0 = overlapping (R[c], R[c+1])
            # pairs of the interleaved prev rowbuf (hand-built strided AP),
            # data1 = (0, d) pairs from the interleaved gather tiles. No
            # separate mud instruction, one fewer dep link per row.
            out_flat = out_d[:].rearrange("q s -> (q s)")
            T2 = 2 * T

            def fused_row_scan(cur_ap, prev_tile, np_, g_ap):
                # data0: [np, 1+2T) viewed as [np, T, 2] with strides (2, 2)
                a = prev_tile[:np_, 1:T2 + 1].copy()
                ap0 = [list(x) for x in a.ap]
                assert ap0[-1][0] == 1 and ap0[-1][1] == T2, ap0
                a.ap = mybir.VecI64Pair(ap0[:-1] + [[2, T], [2, 2]])
                nc.vector.add_instruction(
                    mybir.InstTensorScalarPtr(
                        name=nc.get_next_instruction_name(),
                        is_tensor_tensor_scan=True,
                        is_scalar_tensor_tensor=True,
                        op0=MIN,
                        op1=ADD,
                        ins=[nc.vector.lower_ap(a),
                             mybir.ImmediateValue(dtype=f32, value=BIG),
                             nc.vector.lower_ap(g_ap)],
                        outs=[nc.vector.lower_ap(cur_ap)],
                    )
                )

            for pt, (p0, np_) in enumerate(PT):
                gpool = [gap, gbp, gcp][pt]

                # interleaved rowbufs: [pad, border, (partial, R) * T, pad]
                r_a = dpp.tile([128, T2 + 4], f32, tag=f"ra{pt}")
                r_b = dpp.tile([128, T2 + 4], f32, tag=f"rb{pt}")
                # row 0 of the R-grid lives in r_a's odd slots: corner 0 at
                # the border slot 1, BIG elsewhere; r_b border = BIG.
                # memsets on Pool: keeps them off the DVE critical chain.
                nc.gpsimd.memset(r_a[:np_, 1:T2 + 2], BIG)
                nc.gpsimd.memset(r_a[:np_, 1:2], 0.0)
                nc.gpsimd.memset(r_b[:np_, 1:2], BIG)

                WINS = WINS_S[pt]
                win_of = {}
                for wi, (w0, wn) in enumerate(WINS):
                    for i in range(w0, w0 + wn):
                        win_of[i] = (wi, w0)
                g_tiles = {}
                for i in range(T):
                    wi, w0 = win_of[i]
                    if i == w0:
                        wn = WINS[wi][1]
                        g_t = gpool.tile([128, 16 * T2], bf16, tag=f"g{pt}")
                        g_tiles[wi] = g_t
                        # One DMA per window (full SDMA-engine spread).
                        # Pool/SWDGE: idle sequencer, not paced by ACT/SP.
                        # Window 0 splits off the final query so the earlier
                        # queries prefetch while the last matrix is still
                        # being written.
                        if pt >= 1 and wi == 0:
                            cut = np_ - S
                            nc.gpsimd.dma_start(
                                g_t[:cut, :wn * T2].rearrange(
                                    "p (w j2) -> p w j2", j2=T2),
                                dsc_p[p0:p0 + cut, w0:w0 + wn, :],
                            )
                            nc.gpsimd.dma_start(
                                g_t[cut:np_, :wn * T2].rearrange(
                                    "p (w j2) -> p w j2", j2=T2),
                                dsc_p[p0 + cut:p0 + np_, w0:w0 + wn, :],
                            )
                        else:
                            nc.gpsimd.dma_start(
                                g_t[:np_, :wn * T2].rearrange(
                                    "p (w j2) -> p w j2", j2=T2),
                                dsc_p[p0:p0 + np_, w0:w0 + wn, :],
                            )
                    g_t = g_tiles[wi]
                    prev, cur = (r_a, r_b) if i % 2 == 0 else (r_b, r_a)
                    fused_row_scan(
                        cur[:np_, 2:T2 + 2], prev, np_,
                        g_t[:np_, (i - w0) * T2:(i - w0 + 1) * T2])
                    if i == 0:
                        # row-0 buffer becomes an interior row: border 0 -> BIG
                        nc.vector.memset(prev[:np_, 1:2], BIG)

                final = r_b if T % 2 == 1 else r_a  # T=128 even -> last cur=r_a
                nc.sync.dma_start(
                    out_flat[p0:p0 + np_], final[:np_, T2 + 1:T2 + 2])

    ents = getattr(tc, "_perfetto_entries", None)
    if ents:
        _predicted_ns = int(max(e[2] for e in ents))
    nc.compile()
    return nc


def _pack_inputs(X, Yf):
    """Host-side packing into the exact SBUF layouts the kernel DMAs 1:1."""
    f8 = ml_dtypes.float8_e4m3
    # xt: [QPAD, 128(dk), NKK, 2, T] = fp8(-2*X)^T, DoubleRow slab layout
    # contraction index d = kk*256 + slab*128 + dk
    Xp = np.zeros((QPAD, T, DD), np.float32)
    Xp[:Q] = X
    xtq = np.ascontiguousarray(
        (-2.0 * Xp).astype(f8).transpose(0, 2, 1)        # [QPAD, DD, T]
        .reshape(QPAD, NKK, 2, 128, T).transpose(0, 3, 1, 2, 4))
    # yt: [128(dk), NKK, 2, SJ] = fp8(Y)^T
    yt = np.ascontiguousarray(
        Yf.astype(f8).transpose(2, 0, 1)                 # [DD, S, T]
        .reshape(NKK, 2, 128, SJ).transpose(2, 0, 1, 3))
    # norms folded into fp8 aux DoubleRow rows:
    # x2+y2 = 4096 (exact, 64*64) + rx + ry residuals (fp8/8, err ~+-8)
    x2 = np.einsum("qtd,qtd->qt", Xp, Xp, dtype=np.float32)  # [QPAD, T]
    y2 = np.einsum("std,std->st", Yf, Yf, dtype=np.float32)  # [S, T]
    xa = np.zeros((QPAD, 2, 2, T), f8)
    xa[:, 0, 0, :] = f8(64.0)
    xa[:, 1, 0, :] = ((x2 - 2048.0) / 8.0).astype(f8)
    xa[:, 0, 1, :] = f8(8.0)
    ya = np.zeros((2, 2, SJ), f8)
    ya[0, 0, :] = f8(64.0)
    ya[1, 0, :] = f8(8.0)
    ya[0, 1, :] = ((y2.reshape(SJ) - 2048.0) / 8.0).astype(f8)
    return xtq, yt, xa, ya


def kernel(support_features, support_labels, target_features, n_classes):
    global _built
    from concourse.bass_utils import run_bass_kernel_spmd

    X = np.asarray(target_features, dtype=np.float32)
    Yf = np.asarray(support_features, dtype=np.float32)
    labels = np.asarray(support_labels)
    ncls = int(np.asarray(n_classes))
    assert X.shape == (Q, T, DD) and Yf.shape == (S, T, DD), (
        f"kernel compiled for fixed shapes; got {X.shape}, {Yf.shape}")

    xtq, yt, xa, ya = _pack_inputs(X, Yf)

    if _built is None:
        _built = _build()
    nc = _built

    in_maps = [
        {
            "xt": np.ascontiguousarray(xtq[c * QC:(c + 1) * QC]),
            "yt": yt,
            "xa": np.ascontiguousarray(xa[c * QC:(c + 1) * QC]),
            "ya": ya,
        }
        for c in range(NCORES)
    ]
    res = run_bass_kernel_spmd(nc, in_maps, list(range(NCORES)))
    global _last_result
    _last_result = res
    cum = np.concatenate([res.results[c]["out_cd"] for c in range(NCORES)])[:Q]

    onehot = (labels[:, None] == np.arange(ncls)[None, :]).astype(np.float32)
    counts = np.maximum(onehot.sum(axis=0), 1.0).astype(np.float32)
    logits = -(cum.astype(np.float32) @ onehot) / counts
    return logits.astype(np.float32)


# revision 10
# speedup vs baseline: 2.2465x; 1.0120x over previous
"""Trainium2 Bass kernel: CNN-feature SoftDTW few-shot classifier.

Computes, for Q=100 query sequences and S=25 support sequences (T=128 steps,
D=2048 features): pairwise squared-euclidean cost matrices, soft-DTW alignment
cost per (query, support) pair, then per-class mean distances -> logits.

Key numerical fact: with gamma=0.1 and cost magnitudes ~4096, the reference's
fp32 softmin is bitwise the hard min (exp((m-x)/gamma) underflows for every
non-minimal branch), so the DP is computed with min/add only.

Design (385us baseline -> 174us):
  - fp8e4m3 DoubleRow matmul (0.5 cy/row, K=256/instr) for xy = (-2X)@Y^T:
    4x the bf16 rate. k-outer order, 7 PSUM banks live, so each weight tile
    streams all 3200 columns.
  - x2[i] + y2[s,j] folded into ONE extra fp8 DoubleRow instruction per
    chunk via aux contraction rows: 64*64 (=4096, the exact mean) plus
    (x2-2048)/8 * 8 and 8 * (y2-2048)/8 residuals (+-8 quantization on a
    ~4096 cell; path-sum error ~1e-4 of output scale, threshold 2e-2).
  - DP: ONE fused tensor_tensor_scan per DTW row (free = 2T): the 3-way min
    is expanded into two scan steps per cell,
        state = min(R_prev[c],   state) + 0
        state = min(R_prev[c+1], state) + d_c      (= R_cur[c])
    data0 is a hand-built overlapping strided AP ([j:2,T][slot:2,2]) over
    the interleaved fp32 rowbuf; data1 interleaves zeros with bf16 costs.
    The zero-interleaving rides for free: PSUM evacuation writes odd slots
    of a persistently even-zeroed staging buffer, and the (0,d) pairs flow
    through DRAM and the gather DMA contiguously. This removes the separate
    min(up,diag) instruction AND one dependency link per row: 327ns/row
    engine, 427ns/row serial-chain latency.
  - 3 pair streams (q0-2 | q3-7 | q8-12, pairs on partitions); stage A
    processes slots in order [0-2, 8-12, 3-7] so the two big streams'
    data gates land earliest. Gathers via Pool/SWDGE windows (first window
    8 rows) with per-stream double-buffered pools.

Sharding: data-parallel over queries, 13 per core (Q padded 100->104),
supports replicated. Host: fp8 packing/transposes, x2/y2 sums, final
class-mean logits.
"""

import sys

for _p in ("/opt/trn_rl_repo",):
    if _p not in sys.path:
        sys.path.insert(0, _p)

import numpy as np
import ml_dtypes

# Problem shape (hardcoded: harness runs kernel.py standalone)
Q, S, T, DD = 100, 25, 128, 2048
NCORES = 8
QC = 13                 # queries per core; Q padded to 104
QPAD = QC * NCORES
NKK = DD // 256         # 8 fp8 DoubleRow contraction tiles (256 deep each)
SJ = S * T              # 3200 = flattened (support, j)
B = QC * S              # 325 pairs per core
# DP pair-tile streams aligned to query boundaries (offset, count):
# a stream's first row can run as soon as its LAST query's cost matrix is
# in DRAM. 3/5/5 queries: the DVE wavefront starts at q2 instead of q4.
PT = [(0, 75), (75, 125), (200, 125)]   # q0-2 | q3-7 | q8-12
# stage-A processing order of the 13 query slots. Streams gate on their
# LAST slot: s1 on slot 2, s2 on slot 7, s3 on slot 12.
# Process s3's queries (slots 8-12) right after s1's gate: the wavefront
# stream gates land at positions (3, 13, 8) of 13 - the two big streams
# start as early as possible and the middle one last.
QORDER = [0, 1, 2, 8, 9, 10, 11, 12, 3, 4, 5, 6, 7]
# DP row-windows per gather DMA: small first window so each stream's
# wavefront starts ~2us earlier (less gather gen+transfer to wait on).
# Stream 2 (middle) uses smaller windows + a single-buffered gather pool:
# its rows are RELEASED gradually, so the greedy scheduler cannot drain
# stream 2 early and leave stream 3 alone (at 66% serial rate) in the tail.
def _wins(first, mid):
    w, i = [(0, first)], first
    while i < T:
        n = min(mid, T - i)
        w.append((i, n))
        i += n
    return w[:len(w)]
WINS_S = {0: _wins(8, 16), 1: _wins(8, 16), 2: _wins(8, 16)}
for _w in WINS_S.values():
    assert sum(n for _, n in _w) == T
CH = 512                # matmul moving-chunk / PSUM bank width
# all chunks >=256 so the fp32r rank-2 matmul stays at 1 cycle/row
_CW = [512, 512, 512, 512, 512, 384, 256]
CHUNKS = [(sum(_CW[:i]), w) for i, w in enumerate(_CW)]
assert sum(_CW) == SJ
BIG = 1e10

_built = None          # cached compiled Bass program
_last_result = None    # last BassKernelResults (exec_time_ns when traced)
_predicted_ns = None   # Tile cost-model makespan of the per-core program


def _build():
    import concourse.bacc as bacc
    import concourse.mybir as mybir
    import concourse.tile as tile

    f32 = mybir.dt.float32
    f32r = mybir.dt.float32r
    bf16 = mybir.dt.bfloat16
    fp8 = mybir.dt.float8e4
    MIN = mybir.AluOpType.min
    ADD = mybir.AluOpType.add
    DR = mybir.MatmulPerfMode.DoubleRow

    global _predicted_ns
    nc = bacc.Bacc("TRN2", debug=False)

    xt_d = nc.dram_tensor("xt", [QC, 128, NKK, 2, T], fp8, kind="ExternalInput")
    yt_d = nc.dram_tensor("yt", [128, NKK, 2, SJ], fp8, kind="ExternalInput")
    # aux DoubleRow rows (K_p=2, 2 slabs = 4 contraction rows):
    #   (k0,s0): 64 * 64            -> +4096 (exact mean of x2+y2)
    #   (k1,s0): rx[i]/8 * 8        -> +(x2[i]-2048) residual
    #   (k0,s1): 8 * ry[sj]/8       -> +(y2[sj]-2048) residual
    #   (k1,s1): 0
    xa_d = nc.dram_tensor("xa", [QC, 2, 2, T], fp8, kind="ExternalInput")
    ya_d = nc.dram_tensor("ya", [2, 2, SJ], fp8, kind="ExternalInput")
    out_d = nc.dram_tensor("out_cd", [QC, S], f32, kind="ExternalOutput")
    # cost matrices staged pair-major and ZERO-INTERLEAVED: [q, s, i, 2T]
    # holds (0, d) pairs so the DP can run ONE fused scan per row:
    #   step even: state = min(R_prev[c], state) + 0
    #   step odd:  state = min(R_prev[c+1], state) + d_c   (= R_cur[c])
    dsc = nc.dram_tensor("dsc", [QC, S, T, 2 * T], bf16)
    dsc_p = dsc[:].rearrange("q s i j2 -> (q s) i j2")

    with tile.TileContext(nc) as tc:
        with (
            tc.tile_pool(name="const", bufs=1) as constp,
            tc.tile_pool(name="xq", bufs=3) as xqp,
            tc.tile_pool(name="augq", bufs=3) as augqp,
            tc.tile_pool(name="psum", bufs=8, space="PSUM") as psump,
            tc.tile_pool(name="dq", bufs=1) as dqp,
            tc.tile_pool(name="ga", bufs=2) as gap,
            tc.tile_pool(name="gb", bufs=2) as gbp,
            tc.tile_pool(name="gc", bufs=2) as gcp,
            tc.tile_pool(name="dp", bufs=1) as dpp,
        ):
            # q0's operands first on the ACT queue (ahead of the yt pieces).
            xt0_sb = xqp.tile([128, NKK, 2, T], fp8, tag="xt")
            nc.scalar.dma_start(xt0_sb[:], xt_d[QORDER[0]])
            xa0_sb = augqp.tile([2, 2, T], fp8, tag="xa")
            nc.scalar.dma_start(xa0_sb[:], xa_d[QORDER[0]])

            # Resident Y^T (fp8), per K-tile so q0 starts after ~800KB,
            # spread across 3 DMA queues (SP + ACT + Pool/SWDGE) so all of
            # yt lands within ~7us. The Pool sequencer is idle until the
            # wavefront starts, so the SWDGE generation there is free.
            yt_sb = constp.tile([128, NKK, 2, SJ], fp8)
            for k in range(NKK):
                qeng = [nc.sync, nc.scalar, nc.gpsimd][k % 3]
                qeng.dma_start(yt_sb[:, k, :, :], yt_d[:, k, :, :])
            # ya is only 2 partitions deep (12.8KB per partition = 4.9us on
            # one ring): quarter it across rings so q0's aux chunks aren't
            # gated on one long transfer.
            ya_sb = constp.tile([2, 2, SJ], fp8)
            for yi in range(4):
                c0, c1 = yi * (SJ // 4), (yi + 1) * (SJ // 4)
                qeng = [nc.sync, nc.scalar, nc.gpsimd, nc.sync][yi]
                qeng.dma_start(ya_sb[:, :, c0:c1], ya_d[:, :, c0:c1])

            # Two persistent interleaved staging buffers [128, 2*SJ]:
            # odd slots take the PSUM evacuation (strided ACT writes), even
            # slots are zeroed ONCE here (idle DVE) and persist physically.
            dq_bufs = []
            for di in range(3):
                dq_t = dqp.tile([128, 2 * SJ], bf16, tag=f"dq{di}")
                dq_bufs.append(dq_t)
                dq_ev = dq_t[:].rearrange("p (sj two) -> p two sj", two=2)
                for c0, cw in CHUNKS:
                    nc.vector.memset(dq_ev[:, 0, c0:c0 + cw], 0.0)

            # PE p-state warm-up: the tensor engine needs ~3us of continuous
            # execution to reach 2.4GHz (0.65/1.2GHz below that). Chew junk
            # matmuls on the already-resident xa0 tile into a scratch bank
            # from t~1.3us so q0's real matmuls run at full clock.
            ps_warm = psump.tile([128, CH], f32, tag="ps")
            for wi in range(40):
                nc.tensor.matmul(
                    ps_warm[:, :128],
                    xa0_sb[:, :, :],
                    xa0_sb[:, :, :],
                    start=(wi == 0),
                    stop=(wi == 39),
                    perf_mode=DR,
                )

            # ---- Stage A: cost matrices, one query at a time (k-outer) ----
            xt_tiles = {QORDER[0]: xt0_sb}
            xa_tiles = {QORDER[0]: xa0_sb}

            def prefetch(qn):
                # Pool/SWDGE ring: keeps the SP queue free for the dsc
                # writes that gate each DP stream's first gather.
                xt_sb_n = xqp.tile([128, NKK, 2, T], fp8, tag="xt")
                nc.gpsimd.dma_start(xt_sb_n[:], xt_d[qn])
                xa_sb_n = augqp.tile([2, 2, T], fp8, tag="xa")
                nc.gpsimd.dma_start(xa_sb_n[:], xa_d[qn])
                xt_tiles[qn] = xt_sb_n
                xa_tiles[qn] = xa_sb_n

            # 2-deep lookahead: q1's operands requested with the yt stream
            prefetch(QORDER[1])
            for qi, q in enumerate(QORDER):
                xt_sb, xa_sb = xt_tiles.pop(q), xa_tiles.pop(q)
                dq_sb = dq_bufs[qi % 3]
                dq_odd = dq_sb[:].rearrange("p (sj two) -> p two sj", two=2)
                # k-OUTER: each fp8 weight tile streams all 7 chunks (7 PSUM
                # banks live), so weight loads amortize 7x; for q0 every
                # arriving yt K-tile immediately feeds all chunks, so q0's
                # matrix completes with the prologue.
                pss = []
                for _ci in range(len(CHUNKS)):
                    ps_t = psump.tile([128, CH], f32, tag="ps")
                    pss.append(ps_t)
                # Stream-gate queries run chunk-major (k-inner): each chunk
                # finishes early so its evacuation overlaps the remaining
                # matmuls - the last evac lands ~2.4us sooner, which is on
                # the critical path of the stream's first gather. Other
                # queries stay k-outer (q0's matrix completes while the yt
                # K-tiles stream in).
                gate = True
                if gate:
                    for ci, (c0, cw) in enumerate(CHUNKS):
                        for kk in range(NKK):
                            nc.tensor.matmul(
                                pss[ci][:, :cw],
                                xt_sb[:, kk, :, :],
                                yt_sb[:, kk, :, c0:c0 + cw],
                                start=(kk == 0),
                                stop=False,
                                perf_mode=DR,
                            )
                        nc.tensor.matmul(
                            pss[ci][:, :cw],
                            xa_sb[:, :, :],
                            ya_sb[:, :, c0:c0 + cw],
                            start=False,
                            stop=True,
                            perf_mode=DR,
                        )
                        nc.scalar.copy(
                            dq_odd[:, 1, c0:c0 + cw], pss[ci][:, :cw])
                        nc.sync.dma_start(
                            dsc[q, c0 // T:(c0 + cw) // T]
                            .rearrange("s i j2 -> i s j2"),
                            dq_sb[:, 2 * c0:2 * (c0 + cw)]
                            .rearrange("i (s j2) -> i s j2", j2=2 * T))
                        if ci == 0 and qi + 2 < QC:
                            prefetch(QORDER[qi + 2])
                    continue
                if True:
                    for kk in range(NKK):
                        for ci, (c0, cw) in enumerate(CHUNKS):
                            nc.tensor.matmul(
                                pss[ci][:, :cw],
                                xt_sb[:, kk, :, :],
                                yt_sb[:, kk, :, c0:c0 + cw],
                                start=(kk == 0),
                                stop=False,
                                perf_mode=DR,
                            )
                for ci, (c0, cw) in enumerate(CHUNKS):
                    # fp8 aux rank-4: + 4096 + rx[i] + ry[s,j]
                    nc.tensor.matmul(
                        pss[ci][:, :cw],
                        xa_sb[:, :, :],
                        ya_sb[:, :, c0:c0 + cw],
                        start=False,
                        stop=True,
                        perf_mode=DR,
                    )
                    nc.scalar.copy(dq_odd[:, 1, c0:c0 + cw], pss[ci][:, :cw])
                    if ci == 0 and qi + 2 < QC:
                        # 2-query lookahead on the SP ring, emitted ahead of
                        # this query's dsc writes: the transfer clears the
                        # queue a full query before the PE needs it.
                        prefetch(QORDER[qi + 2])
                    # per-chunk dsc write (chunk widths are whole s-blocks)
                    nc.sync.dma_start(
                        dsc[q, c0 // T:(c0 + cw) // T]
                        .rearrange("s i j2 -> i s j2"),
                        dq_sb[:, 2 * c0:2 * (c0 + cw)]
                        .rearrange("i (s j2) -> i s j2", j2=2 * T))

            # ---- Stage B: hard-DTW wavefront, 3 batched pair-tiles ----
            # One FUSED scan per row (free = 2T): the 3-way min is expanded
            # into two scan steps per cell, data0 = overlapping (R[c], R[c+1])
            # pairs of the interleaved prev rowbuf (hand-built strided AP),
            # data1 = (0, d) pairs from the interleaved gather tiles. No
            # separate mud instruction, one fewer dep link per row.
            out_flat = out_d[:].rearrange("q s -> (q s)")
            T2 = 2 * T

            def fused_row_scan(cur_ap, prev_tile, np_, g_ap):
                # data0: [np, 1+2T) viewed as [np, T, 2] with strides (2, 2)
                a = prev_tile[:np_, 1:T2 + 1].copy()
                ap0 = [list(x) for x in a.ap]
                assert ap0[-1][0] == 1 and ap0[-1][1] == T2, ap0
                a.ap = mybir.VecI64Pair(ap0[:-1] + [[2, T], [2, 2]])
                nc.vector.add_instruction(
                    mybir.InstTensorScalarPtr(
                        name=nc.get_next_instruction_name(),
                        is_tensor_tensor_scan=True,
                        is_scalar_tensor_tensor=True,
                        op0=MIN,
                        op1=ADD,
                        ins=[nc.vector.lower_ap(a),
                             mybir.ImmediateValue(dtype=f32, value=BIG),
                             nc.vector.lower_ap(g_ap)],
                        outs=[nc.vector.lower_ap(cur_ap)],
                    )
                )

            for pt, (p0, np_) in enumerate(PT):
                gpool = [gap, gbp, gcp][pt]

                # interleaved rowbufs: [pad, border, (partial, R) * T, pad]
                r_a = dpp.tile([128, T2 + 4], f32, tag=f"ra{pt}")
                r_b = dpp.tile([128, T2 + 4], f32, tag=f"rb{pt}")
                # row 0 of the R-grid lives in r_a's odd slots: corner 0 at
                # the border slot 1, BIG elsewhere; r_b border = BIG.
                # memsets on Pool: keeps them off the DVE critical chain.
                nc.gpsimd.memset(r_a[:np_, 1:T2 + 2], BIG)
                nc.gpsimd.memset(r_a[:np_, 1:2], 0.0)
                nc.gpsimd.memset(r_b[:np_, 1:2], BIG)

                WINS = WINS_S[pt]
                win_of = {}
                for wi, (w0, wn) in enumerate(WINS):
                    for i in range(w0, w0 + wn):
                        win_of[i] = (wi, w0)
                g_tiles = {}
                for i in range(T):
                    wi, w0 = win_of[i]
                    if i == w0:
                        wn = WINS[wi][1]
                        g_t = gpool.tile([128, 16 * T2], bf16, tag=f"g{pt}")
                        g_tiles[wi] = g_t
                        # One DMA per window (full SDMA-engine spread).
                        # Pool/SWDGE: idle sequencer, not paced by ACT/SP.
                        # Window 0 splits off the final query so the earlier
                        # queries prefetch while the last matrix is still
                        # being written.
                        if pt >= 1 and wi == 0:
                            cut = np_ - S
                            nc.gpsimd.dma_start(
                                g_t[:cut, :wn * T2].rearrange(
                                    "p (w j2) -> p w j2", j2=T2),
                                dsc_p[p0:p0 + cut, w0:w0 + wn, :],
                            )
                            nc.gpsimd.dma_start(
                                g_t[cut:np_, :wn * T2].rearrange(
                                    "p (w j2) -> p w j2", j2=T2),
                                dsc_p[p0 + cut:p0 + np_, w0:w0 + wn, :],
                            )
                        else:
                            nc.gpsimd.dma_start(
                                g_t[:np_, :wn * T2].rearrange(
                                    "p (w j2) -> p w j2", j2=T2),
                                dsc_p[p0:p0 + np_, w0:w0 + wn, :],
                            )
                    g_t = g_tiles[wi]
                    prev, cur = (r_a, r_b) if i % 2 == 0 else (r_b, r_a)
                    fused_row_scan(
                        cur[:np_, 2:T2 + 2], prev, np_,
                        g_t[:np_, (i - w0) * T2:(i - w0 + 1) * T2])
                    if i == 0:
                        # row-0 buffer becomes an interior row: border 0 -> BIG
                        nc.vector.memset(prev[:np_, 1:2], BIG)

                final = r_b if T % 2 == 1 else r_a  # T=128 even -> last cur=r_a
                nc.sync.dma_start(
                    out_flat[p0:p0 + np_], final[:np_, T2 + 1:T2 + 2])

    ents = getattr(tc, "_perfetto_entries", None)
    if ents:
        _predicted_ns = int(max(e[2] for e in ents))
    nc.compile()
    return nc


def _pack_inputs(X, Yf):
    """Host-side packing into the exact SBUF layouts the kernel DMAs 1:1."""
    f8 = ml_dtypes.float8_e4m3
    # xt: [QPAD, 128(dk), NKK, 2, T] = fp8(-2*X)^T, DoubleRow slab layout
    # contraction index d = kk*256 + slab*128 + dk
    Xp = np.zeros((QPAD, T, DD), np.float32)
    Xp[:Q] = X
    xtq = np.ascontiguousarray(
        (-2.0 * Xp).astype(f8).transpose(0, 2, 1)        # [QPAD, DD, T]
        .reshape(QPAD, NKK, 2, 128, T).transpose(0, 3, 1, 2, 4))
    # yt: [128(dk), NKK, 2, SJ] = fp8(Y)^T
    yt = np.ascontiguousarray(
        Yf.astype(f8).transpose(2, 0, 1)                 # [DD, S, T]
        .reshape(NKK, 2, 128, SJ).transpose(2, 0, 1, 3))
    # norms folded into fp8 aux DoubleRow rows:
    # x2+y2 = 4096 (exact, 64*64) + rx + ry residuals (fp8/8, err ~+-8)
    x2 = np.einsum("qtd,qtd->qt", Xp, Xp, dtype=np.float32)  # [QPAD, T]
    y2 = np.einsum("std,std->st", Yf, Yf, dtype=np.float32)  # [S, T]
    xa = np.zeros((QPAD, 2, 2, T), f8)
    xa[:, 0, 0, :] = f8(64.0)
    xa[:, 1, 0, :] = ((x2 - 2048.0) / 8.0).astype(f8)
    xa[:, 0, 1, :] = f8(8.0)
    ya = np.zeros((2, 2, SJ), f8)
    ya[0, 0, :] = f8(64.0)
    ya[1, 0, :] = f8(8.0)
    ya[0, 1, :] = ((y2.reshape(SJ) - 2048.0) / 8.0).astype(f8)
    return xtq, yt, xa, ya


def kernel(support_features, support_labels, target_features, n_classes):
    global _built
    from concourse.bass_utils import run_bass_kernel_spmd

    X = np.asarray(target_features, dtype=np.float32)
    Yf = np.asarray(support_features, dtype=np.float32)
    labels = np.asarray(support_labels)
    ncls = int(np.asarray(n_classes))
    assert X.shape == (Q, T, DD) and Yf.shape == (S, T, DD), (
        f"kernel compiled for fixed shapes; got {X.shape}, {Yf.shape}")

    xtq, yt, xa, ya = _pack_inputs(X, Yf)

    if _built is None:
        _built = _build()
    nc = _built

    in_maps = [
        {
            "xt": np.ascontiguousarray(xtq[c * QC:(c + 1) * QC]),
            "yt": yt,
            "xa": np.ascontiguousarray(xa[c * QC:(c + 1) * QC]),
            "ya": ya,
        }
        for c in range(NCORES)
    ]
    res = run_bass_kernel_spmd(nc, in_maps, list(range(NCORES)))
    global _last_result
    _last_result = res
    cum = np.concatenate([res.results[c]["out_cd"] for c in range(NCORES)])[:Q]

    onehot = (labels[:, None] == np.arange(ncls)[None, :]).astype(np.float32)
    counts = np.maximum(onehot.sum(axis=0), 1.0).astype(np.float32)
    logits = -(cum.astype(np.float32) @ onehot) / counts
    return logits.astype(np.float32)
